# revision 61
# baseline (speedup 1.0000x reference)
"""BitSwiGLU Trainium2 kernel — tensor-parallel over hidden, 8 NeuronCores.

Math (per bit_linear, forward values):
    gamma_x = clip(max|x_row|, 1e-5);  k = rne(x * 127/gamma_x)  in [-127,127]
    gamma_w = clip(mean|w|, 1e-5);    t = sign(w) * (|w| > 0.5*gamma_w)
    y = (k @ t.T) * (gamma_x*gamma_w/127) + b

k and t are small integers, exactly representable in bf16; the TensorEngine
accumulates bf16 products in fp32 PSUM, so k @ t.T is EXACT integer math at
bf16 speed. Ternarization runs as t2 = sign(w-thr)+sign(w+thr) in {-2,0,2};
the factor 2 is folded into the eviction scales.

v4 design notes (driven by the v3 trace):
 - The HAM trace showed the PE clock held at K=13/16 (1.95 GHz) through
   mm1 while M2 (PE-only) ran at 8/8 (2.4 GHz): the chip power budget
   throttles the PE when VectorE/ScalarE/DMA run hot alongside it. So
   redundant work is not free even when perfectly overlapped.
 - x-quant is therefore SHARDED: the host hands each core only its own
   1024 tokens (x_own, same interleaving as the output). Each core
   quantizes 1 of 8 token-chunks per block and the integer kxT slabs are
   AllGather-ed (0.5MB -> 4MB per block, triggered TWO blocks early so
   the collective stream has slack). gamma_x rides along as a bf16
   hi/lo pair in 2 extra columns (rel err 2^-17; only the eviction
   scales consume it — the owner core quantizes with exact f32).
   This cuts per-core mm1 DVE+ACT work ~2x and HBM reads 8x on x.
 - mm2 lhsT (khT) is assembled in M2 directly from the AllToAll output
   with transpose-DMAs into SBUF group tiles (no DRAM round trip, no
   mm1-time transposes): M2 has both SBUF room and power headroom.
 - h stays in SBUF (hbuf, 2 block buffers); only block 7 spills (its
   requant runs in M2 scope). requant_block(r) emits at block r+1 qb3
   so its gamma_h AllReduce(max) wait is satisfied before it reaches
   the strict-FIFO queues. Rounding steps run on ScalarE.
 - out_w ternarize is spread thin over blocks 0..5 (two 1MB batches per
   block); its 32MB AllGather triggers after AR(5) — the one spot where
   the single collective stream has a ~2-block quiet window.
 - mm2 runs dcol-PAIR passes over two 4-block groups: 8 PSUM banks, one
   2KB weight tile per 8 matmuls so the weight stream keeps pace.

Token ownership: core i owns global tokens {r*1024 + i*128 + [0,128) for
r in 0..7}; the host wrapper slices x_own the same way and re-interleaves
the 8 per-core outputs.
"""

import numpy as np

import concourse.mybir as mybir
import concourse.tile as tile
from concourse import bacc
from concourse import bass_isa
from concourse.bass_utils import run_bass_kernel_spmd

F32 = mybir.dt.float32
BF16 = mybir.dt.bfloat16
AF = mybir.ActivationFunctionType
OP = mybir.AluOpType
AX = mybir.AxisListType

MAGIC = 12582912.0  # 1.5 * 2**23 : (v + MAGIC) - MAGIC == rne(v) for |v| < 2**22

N_CORES = 8
D = 2048            # d_model
H = 8192            # hidden (full)
HL = H // N_CORES   # 1024 hidden per core
T = 8192            # total tokens
TL = T // N_CORES   # 1024 tokens owned per core
KD = D // 128       # 16 contraction chunks, mm1
KHL = HL // 128     # 8  chunks of the local hidden slice
KH = H // 128       # 64 contraction chunks, mm2
RLAST = N_CORES - 1
RG = [list(range(N_CORES))]
AGW = KD * 128 + 2  # kx AllGather row width: kxT flat + gamma_x hi/lo


def _build():
    nc = bacc.Bacc("TRN2", target_bir_lowering=False, debug=False,
                   num_devices=N_CORES)
    xo_d = nc.dram_tensor("x_own", [TL, D], F32, kind="ExternalInput")
    gwT_d = nc.dram_tensor("gwT", [D, HL], F32, kind="ExternalInput")
    vwT_d = nc.dram_tensor("vwT", [D, HL], F32, kind="ExternalInput")
    owT_d = nc.dram_tensor("owT", [HL, D], F32, kind="ExternalInput")
    sel_d = nc.dram_tensor("sel8", [1, N_CORES], F32, kind="ExternalInput")
    out_d = nc.dram_tensor("out", [TL, D], F32, kind="ExternalOutput")

    with tile.TileContext(nc) as tc:
        _body(tc, xo_d, gwT_d, vwT_d, owT_d, sel_d, out_d)
    nc.compile()
    return nc


def _body(tc, xo_d, gwT_d, vwT_d, owT_d, sel_d, out_d):
    nc = tc.nc
    gp = nc.gpsimd

    with (
        tc.tile_pool(name="pp", bufs=1) as pp,
        tc.tile_pool(name="psp", bufs=8, space="PSUM") as psp,
        tc.tile_pool(name="drp", bufs=1, space="DRAM") as drp,
    ):
        # ---------- DRAM scratch ----------
        gv_i = drp.tile([1, 4], F32, tag="gv_i")
        gv_o = drp.tile([1, 4], F32, tag="gv_o", addr_space="Shared")
        go_i = drp.tile([1, 4], F32, tag="go_i")
        go_o = drp.tile([1, 4], F32, tag="go_o", addr_space="Shared")
        # ternary out_w travels as INT8 ({-2,0,2} exact): halves the
        # AllGather wire AND mm2's weight-stream HBM draw; VectorE (idle
        # in M2) casts tiles back to bf16 for the PE. AG in 4 chunks of
        # 256 rows/core: chunk q holds rows i*1024 + q*256 + [0,256)
        I8 = mybir.dt.int8
        w2b = drp.tile([HL, D], I8, tag="w2b")
        w2gq = [drp.tile([N_CORES * 256, D], I8, tag=f"w2gq{q}",
                         name=f"w2gq{q}", addr_space="Shared")
                for q in range(4)]
        arh_i = [drp.tile([1, TL], F32, tag=f"arhi{r}", name=f"arhi{r}")
                 for r in range(N_CORES)]
        arh_o = [drp.tile([1, TL], F32, tag=f"arho{r}", name=f"arho{r}",
                          addr_space="Shared") for r in range(N_CORES)]
        a2i = [drp.tile([N_CORES, 128, HL], BF16, tag=f"a2i{r}",
                        name=f"a2i{r}") for r in range(N_CORES)]
        a2o = [drp.tile([N_CORES, 128, HL], BF16, tag=f"a2o{r}",
                        name=f"a2o{r}") for r in range(N_CORES)]
        agx_i = [drp.tile([128, AGW], BF16, tag=f"agxi{r}",
                          name=f"agxi{r}") for r in range(N_CORES)]
        agx_o = [drp.tile([N_CORES, 128, AGW], BF16, tag=f"agxo{r}",
                          name=f"agxo{r}", addr_space="Shared")
                 for r in range(N_CORES)]

        # ---------- persistent SBUF (whole program) ----------
        gam = pp.tile([128, 4], F32, tag="gam")             # g, v, o gammas
        thr = pp.tile([128, 6], F32, tag="thr")             # +-thr g/v/o
        gxall = pp.tile([128, 64], F32, tag="gxall")        # gamma_x per tok
        s1a = pp.tile([128, 64], F32, tag="s1a")
        s2a = pp.tile([128, 64], F32, tag="s2a")
        s12a = pp.tile([128, 64], F32, tag="s12a")
        selb = pp.tile([128, N_CORES], F32, tag="selb")
        sofull = [pp.tile([128, KHL], F32, tag=f"sofull{r}",
                          name=f"sofull{r}") for r in range(N_CORES)]
        sosel = [pp.tile([128, 1], F32, tag=f"sosel{r}", name=f"sosel{r}")
                 for r in range(N_CORES)]

        Gv = gwT_d.ap().rearrange("(c p) h -> c p h", p=128)    # 16 x [128,HL]
        Vv = vwT_d.ap().rearrange("(c p) h -> c p h", p=128)
        Ov = owT_d.ap().rearrange("(c p) d -> c p d", p=128)    # 8 x [128,D]
        Xo = xo_d.ap().rearrange("(r p) d -> r p d", p=128)     # 8 x [128,D]

        thr_g, nthr_g = thr[:, 0:1], thr[:, 1:2]
        thr_v, nthr_v = thr[:, 2:3], thr[:, 3:4]
        thr_o, nthr_o = thr[:, 4:5], thr[:, 5:6]

        def tern_act(pool, wt_ap, out_ap, thr_p, thr_n, w=HL):
            # bufs=4: the ScalarE Sign stream must not be throttled by
            # the VectorE adds' buffer returns (vector runs a backlog
            # during the prologue ternarize)
            sp = pool.tile([128, w], BF16, tag="q_sp", bufs=3)
            nc.scalar.activation(out=sp[:, :], in_=wt_ap,
                                 func=AF.Sign, bias=thr_n)
            sn = pool.tile([128, w], BF16, tag="q_sn", bufs=3)
            nc.scalar.activation(out=sn[:, :], in_=wt_ap,
                                 func=AF.Sign, bias=thr_p)
            nc.vector.tensor_add(out=out_ap, in0=sp[:, :], in1=sn[:, :])

        def tern_dve(pool, wt_ap, out_ap, thr_p, thr_n, w=HL):
            mp = pool.tile([128, w], BF16, tag="q_sp", bufs=3)
            nc.vector.tensor_scalar(out=mp[:, :], in0=wt_ap,
                                    scalar1=thr_p, scalar2=2.0,
                                    op0=OP.is_gt, op1=OP.mult)
            mn = pool.tile([128, w], BF16, tag="q_sn", bufs=3)
            nc.vector.tensor_scalar(out=mn[:, :], in0=wt_ap,
                                    scalar1=thr_n, scalar2=2.0,
                                    op0=OP.is_lt, op1=OP.mult)
            nc.vector.tensor_sub(out=out_ap, in0=mp[:, :], in1=mn[:, :])

        # rqs: tiny per-r requant scratch, lives through M2
        with tc.tile_pool(name="rqs", bufs=1) as rqs:
            ghr = [rqs.tile([128, KHL], F32, tag=f"ghr{r}", name=f"ghr{r}")
                   for r in range(N_CORES)]
            shr = [rqs.tile([128, KHL], F32, tag=f"shr{r}", name=f"shr{r}")
                   for r in range(N_CORES)]

            def requant_scales(r, pool):
                """AllReduced per-token max -> eviction + requant scales."""
                nc.sync.dma_start(
                    out=ghr[r][:, :],
                    in_=arh_o[r][0, :].rearrange("(ml p) -> p ml", p=128))
                gcl = pool.tile([128, KHL], F32, tag="gcl")
                nc.vector.tensor_scalar_max(out=gcl[:, :], in0=ghr[r][:, :],
                                            scalar1=1e-5)
                nc.vector.tensor_scalar(out=sofull[r][:, :], in0=gcl[:, :],
                                        scalar1=gam[:, 2:3],
                                        scalar2=1.0 / 254.0,
                                        op0=OP.mult, op1=OP.mult)
                solm = pool.tile([128, KHL], F32, tag="solm")
                nc.vector.tensor_mul(out=solm[:, :], in0=sofull[r][:, :],
                                     in1=selb[:, :])
                nc.vector.tensor_reduce(out=sosel[r][:, :], in_=solm[:, :],
                                        axis=AX.X, op=OP.add)
                rcph = pool.tile([128, KHL], F32, tag="rcph")
                nc.vector.reciprocal(out=rcph[:, :], in_=gcl[:, :])
                nc.vector.tensor_scalar_mul(out=shr[r][:, :], in0=rcph[:, :],
                                            scalar1=127.0)

            def requant_emit(r, pool, h_slice, on_vector=False):
                """Quantize h block r to bf16 integer levels + AllToAll.
                h_slice(ml, hf) -> [128,512] AP of h (f32). Rounding runs
                on ScalarE during mm1 (VectorE is hotter there) but on
                VectorE for the block-7 instance in M2 (ScalarE is busy
                with khT transposes there)."""
                requant_scales(r, pool)
                for ml in range(KHL):
                    for hf in range(2):
                        hmg = pool.tile([128, HL // 2], F32, tag="hmg")
                        kh = pool.tile([128, HL // 2], BF16, tag="kh")
                        if on_vector:
                            # block-7 instance, runs at the M1->M2 seam:
                            # GpSimd (Pool) does the rounding so neither
                            # the ScalarE khT-transpose stream nor the
                            # VectorE w2t-cast stream is head-of-line
                            # blocked behind the AR(7) wait; the a2i
                            # writes chain on the same queue
                            gp.tensor_scalar(
                                out=hmg[:, :], in0=h_slice(ml, hf),
                                scalar1=shr[r][:, ml:ml + 1],
                                scalar2=MAGIC, op0=OP.mult, op1=OP.add)
                            gp.tensor_scalar_sub(
                                out=kh[:, :], in0=hmg[:, :], scalar1=MAGIC)
                        else:
                            nc.scalar.activation(
                                out=hmg[:, :], in_=h_slice(ml, hf),
                                func=AF.Copy,
                                scale=shr[r][:, ml:ml + 1], bias=MAGIC)
                            nc.scalar.activation(
                                out=kh[:, :], in_=hmg[:, :],
                                func=AF.Copy, bias=-MAGIC)
                        gp.dma_start(
                            out=a2i[r][ml, :,
                                       hf * (HL // 2):(hf + 1) * (HL // 2)],
                            in_=kh[:, :])
                gp.collective_compute("AllToAll", OP.bypass,
                                      replica_groups=RG,
                                      ins=[a2i[r][:, :, :].opt()],
                                      outs=[a2o[r][:, :, :].opt()])

            # =============== prologue ===============
            with (
                tc.tile_pool(name="wW", bufs=1) as wW,
                tc.tile_pool(name="kxp", bufs=2) as kxp,
                tc.tile_pool(name="xq", bufs=2) as xq,
            ):
                def quant_own(r):
                    """Quantize this core's 128-token chunk of block r,
                    pack kxT + gamma_x(hi/lo bf16) into agx_i[r]."""
                    xt = xq.tile([128, D], F32, tag="x_in")
                    nc.sync.dma_start(out=xt[:, :], in_=Xo[r])
                    gxo = xq.tile([128, 1], F32, tag="gxo")
                    gmx = xq.tile([128, 1], F32, tag="gmx")
                    nc.vector.tensor_reduce(out=gmx[:, :], in_=xt[:, :],
                                            axis=AX.X, op=OP.max,
                                            apply_absolute_value=True)
                    nc.vector.tensor_scalar_max(out=gxo[:, :],
                                                in0=gmx[:, :], scalar1=1e-5)
                    rcp = xq.tile([128, 1], F32, tag="rcpx")
                    nc.vector.reciprocal(out=rcp[:, :], in_=gxo[:, :])
                    sx = xq.tile([128, 1], F32, tag="sx")
                    nc.vector.tensor_scalar_mul(out=sx[:, :], in0=rcp[:, :],
                                                scalar1=127.0)
                    nc.vector.tensor_scalar(out=xt[:, :], in0=xt[:, :],
                                            scalar1=sx[:, :], scalar2=MAGIC,
                                            op0=OP.mult, op1=OP.add)
                    kx = xq.tile([128, D], BF16, tag="kx", bufs=1)
                    nc.scalar.activation(out=kx[:, :], in_=xt[:, :],
                                         func=AF.Copy, bias=-MAGIC)
                    kxT = xq.tile([128, KD, 128], BF16, tag="kxT")
                    nc.scalar.dma_start(out=kxT[:, :, :], in_=kx[:, :],
                                        transpose=True)
                    sc2 = xq.tile([128, 2], BF16, tag="sc2")
                    nc.vector.tensor_scalar_add(out=sc2[:, 0:1],
                                                in0=gxo[:, :], scalar1=0.0)
                    nc.vector.tensor_sub(out=sc2[:, 1:2], in0=gxo[:, :],
                                         in1=sc2[:, 0:1])
                    gp.dma_start(
                        out=agx_i[r][:, 0:KD * 128],
                        in_=kxT[:, :, :].rearrange("p k t -> p (k t)"))
                    gp.dma_start(out=agx_i[r][:, KD * 128:AGW],
                                 in_=sc2[:, :])
                    gp.collective_compute("AllGather", OP.bypass,
                                          replica_groups=RG,
                                          ins=[agx_i[r][:, :].opt()],
                                          outs=[agx_o[r][:, :, :].opt()])

                if True:
                    WgT = wW.tile([128, KD, HL], BF16, tag="WgT")   # 4.2 MB
                    WvT = wW.tile([128, KD, HL], BF16, tag="WvT")   # 4.2 MB
                    with tc.tile_pool(name="stg", bufs=2) as stg:
                        Gst = stg.tile([128, KD, HL], F32, tag="Gst",
                                       bufs=1)                      # 8.4 MB
                        parts = stg.tile([128, 4 * KD], F32, tag="parts",
                                         bufs=1)
                        # gate: load + stage + |w|-accum on ScalarE;
                        # val: rotating load + |w|-reduce on VectorE
                        for c in range(KD):
                            nc.sync.dma_start(out=Gst[:, c, :], in_=Gv[c])
                            scr = stg.tile([128, HL], F32, tag="scr", bufs=1)
                            nc.scalar.activation(
                                out=scr[:, :], in_=Gst[:, c, :],
                                func=AF.Abs,
                                accum_out=parts[:, c:c + 1])
                            wt = stg.tile([128, HL], F32, tag="v_in",
                                          bufs=4)
                            nc.sync.dma_start(out=wt[:, :], in_=Vv[c])
                            nc.vector.tensor_reduce(
                                out=parts[:, KD + c:KD + c + 1],
                                in_=wt[:, :], axis=AX.X, op=OP.add,
                                apply_absolute_value=True)
                        gsum = stg.tile([128, 4], F32, tag="gsum", bufs=1)
                        nc.vector.memset(gsum[:, :], 0.0)
                        for j, sl in enumerate((slice(0, KD),
                                                slice(KD, 2 * KD))):
                            red = stg.tile([128, 1], F32, tag="red")
                            nc.vector.tensor_reduce(out=red[:, :],
                                                    in_=parts[:, sl],
                                                    axis=AX.X, op=OP.add)
                            gp.partition_all_reduce(gsum[:, j:j + 1],
                                                    red[:, :], 128,
                                                    bass_isa.ReduceOp.add)
                        nc.sync.dma_start(out=gv_i[0:1, :],
                                          in_=gsum[0:1, :])
                        gp.collective_compute("AllReduce", OP.add,
                                              replica_groups=RG,
                                              ins=[gv_i[:, :].opt()],
                                              outs=[gv_o[:, :].opt()])
                        # own-token quant for blocks 0,1 + their kx
                        # AllGathers ride right behind AllReduce #1
                        quant_own(0)
                        quant_own(1)
                        # out_w |w|-sum pass rides under AllReduce #1
                        for c in range(KHL):
                            for hf in range(2):
                                wt = stg.tile([128, HL], F32, tag="v_in",
                                              bufs=4)
                                nc.sync.dma_start(
                                    out=wt[:, :],
                                    in_=Ov[c][:, hf * HL:(hf + 1) * HL])
                                col = 2 * KD + 2 * c + hf
                                if hf == 0:
                                    scr = stg.tile([128, HL], F32,
                                                   tag="scr", bufs=1)
                                    nc.scalar.activation(
                                        out=scr[:, :], in_=wt[:, :],
                                        func=AF.Abs,
                                        accum_out=parts[:, col:col + 1])
                                else:
                                    nc.vector.tensor_reduce(
                                        out=parts[:, col:col + 1],
                                        in_=wt[:, :], axis=AX.X, op=OP.add,
                                        apply_absolute_value=True)
                        redo = stg.tile([128, 1], F32, tag="red")
                        nc.vector.tensor_reduce(
                            out=redo[:, :],
                            in_=parts[:, 2 * KD:2 * KD + 2 * KHL],
                            axis=AX.X, op=OP.add)
                        gp.partition_all_reduce(gsum[:, 2:3], redo[:, :],
                                                128, bass_isa.ReduceOp.add)
                        nc.sync.dma_start(out=go_i[0:1, :],
                                          in_=gsum[0:1, :])
                        gp.collective_compute("AllReduce", OP.add,
                                              replica_groups=RG,
                                              ins=[go_i[:, :].opt()],
                                              outs=[go_o[:, :].opt()])
                        # gammas g/v from AllReduce #1
                        g0 = stg.tile([1, 4], F32, tag="g0", bufs=1)
                        nc.sync.dma_start(out=g0[:, :], in_=gv_o[0:1, :])
                        gbc = stg.tile([128, 4], F32, tag="gbc", bufs=1)
                        gp.partition_broadcast(gbc[:, :], g0[:, :])
                        nc.vector.tensor_scalar(out=gam[:, 0:2],
                                                in0=gbc[:, 0:2],
                                                scalar1=1.0 / (H * D),
                                                scalar2=1e-5,
                                                op0=OP.mult, op1=OP.max)
                        for j in range(2):
                            nc.vector.tensor_scalar_mul(
                                out=thr[:, 2 * j:2 * j + 1],
                                in0=gam[:, j:j + 1], scalar1=0.5)
                            nc.vector.tensor_scalar_mul(
                                out=thr[:, 2 * j + 1:2 * j + 2],
                                in0=gam[:, j:j + 1], scalar1=-0.5)
                        s0 = stg.tile([1, N_CORES], F32, tag="s0", bufs=1)
                        nc.sync.dma_start(out=s0[:, :], in_=sel_d.ap())
                        gp.partition_broadcast(selb[:, :], s0[:, :])
                        # ternarize: gate from SBUF (ScalarE), val re-read
                        # (GpSimd queue) + ternarize on VectorE
                        for c in range(KD):
                            tern_act(stg, Gst[:, c, :], WgT[:, c, :],
                                     thr_g, nthr_g)
                            wtv = stg.tile([128, HL], F32, tag="v_in",
                                           bufs=4)
                            nc.sync.dma_start(out=wtv[:, :], in_=Vv[c])
                            tern_dve(stg, wtv[:, :], WvT[:, c, :],
                                     thr_v, nthr_v)

                    def ow_gamma_emit(pool):
                        """gamma_o + thresholds from AllReduce #2."""
                        g1 = pool.tile([1, 4], F32, tag="g1", bufs=1)
                        nc.sync.dma_start(out=g1[:, :], in_=go_o[0:1, :])
                        gb1 = pool.tile([128, 4], F32, tag="gb1", bufs=1)
                        gp.partition_broadcast(gb1[:, :], g1[:, :])
                        nc.vector.tensor_scalar(out=gam[:, 2:3],
                                                in0=gb1[:, 2:3],
                                                scalar1=1.0 / (H * D),
                                                scalar2=1e-5,
                                                op0=OP.mult, op1=OP.max)
                        nc.vector.tensor_scalar_mul(out=thr[:, 4:5],
                                                    in0=gam[:, 2:3],
                                                    scalar1=0.5)
                        nc.vector.tensor_scalar_mul(out=thr[:, 5:6],
                                                    in0=gam[:, 2:3],
                                                    scalar1=-0.5)

                    def ow_tern_batch(pool, c):
                        """Ternarize one of out_w's 8 row-chunks (spread
                        across mm1 blocks, two batches per block)."""
                        wts = []
                        for qf in range(4):
                            wt = pool.tile([128, 512], F32, tag="ow_in",
                                           bufs=4)
                            gp.dma_start(
                                out=wt[:, :],
                                in_=Ov[c][:, qf * 512:(qf + 1) * 512])
                            wts.append(wt)
                        for qf in range(4):
                            tq = pool.tile([128, 512], mybir.dt.int8,
                                           tag="ow_tq")
                            tern = tern_act if qf % 2 == 0 else tern_dve
                            tern(pool, wts[qf][:, :], tq[:, :], thr_o,
                                 nthr_o, w=512)
                            gp.dma_start(
                                out=w2b[c * 128:(c + 1) * 128,
                                        qf * 512:(qf + 1) * 512],
                                in_=tq[:, :])

                    # ===== phase M1: mm1 + silu + requant + A2A =====
                    with (
                        tc.tile_pool(name="m1e", bufs=2) as m1e,
                        tc.tile_pool(name="hbp", bufs=1) as hbp,
                        tc.tile_pool(name="rqm", bufs=2) as rqm,
                    ):
                        hbuf = [hbp.tile([128, KHL, HL], F32,
                                         tag=f"hbuf{b}", name=f"hbuf{b}")
                                for b in range(2)]

                        def h_sb(r):
                            def sl(ml, hf):
                                return hbuf[r % 2][:, ml,
                                                   hf * 512:(hf + 1) * 512]
                            return sl

                        for r in range(N_CORES):
                            hmall = m1e.tile([128, KHL], F32, tag="hmall",
                                             bufs=2, name=f"hmall{r}")
                            for qb in range(4):     # 256-token quarters
                                kxq = kxp.tile([128, KD, 256], BF16,
                                               tag="kxq")
                                for j in range(2):
                                    jj = qb * 2 + j
                                    nc.sync.dma_start(
                                        out=kxq[:, :,
                                                j * 128:(j + 1) * 128],
                                        in_=agx_o[r][jj, :, 0:KD * 128]
                                        .rearrange("p (k t) -> p k t",
                                                   t=128))
                                if qb == 0:
                                    if r + 2 < N_CORES:
                                        quant_own(r + 2)
                                    # reconstruct gamma_x + eviction
                                    # scales for the whole block
                                    gxsc = xq.tile([128, KHL, 2], BF16,
                                                   tag="gxsc")
                                    nc.sync.dma_start(
                                        out=gxsc[:, :, :],
                                        in_=agx_o[r][:, :, KD * 128:AGW]
                                        .rearrange("j p c -> p j c"))
                                    r8 = r * 8
                                    nc.vector.tensor_add(
                                        out=gxall[:, r8:r8 + 8],
                                        in0=gxsc[:, :, 0],
                                        in1=gxsc[:, :, 1])
                                    nc.vector.tensor_scalar(
                                        out=s1a[:, r8:r8 + 8],
                                        in0=gxall[:, r8:r8 + 8],
                                        scalar1=gam[:, 0:1],
                                        scalar2=1.0 / 254.0,
                                        op0=OP.mult, op1=OP.mult)
                                    nc.vector.tensor_scalar(
                                        out=s2a[:, r8:r8 + 8],
                                        in0=gxall[:, r8:r8 + 8],
                                        scalar1=gam[:, 1:2],
                                        scalar2=1.0 / 254.0,
                                        op0=OP.mult, op1=OP.mult)
                                    nc.vector.tensor_mul(
                                        out=s12a[:, r8:r8 + 8],
                                        in0=s1a[:, r8:r8 + 8],
                                        in1=s2a[:, r8:r8 + 8])
                                for j in range(2):
                                    ml = qb * 2 + j
                                    m = r * 8 + ml
                                    hm2 = m1e.tile([128, 2], F32,
                                                   tag="hm2")
                                    ps = [psp.tile([128, 512], F32,
                                                   tag="ps",
                                                   name=f"ps{m}_{i}")
                                          for i in range(4)]
                                    for k in range(KD):
                                        lhsT = kxq[:, k,
                                                   j * 128:(j + 1) * 128]
                                        for i, (w, n) in enumerate(
                                                ((WgT, 0), (WvT, 0),
                                                 (WgT, 1), (WvT, 1))):
                                            nc.tensor.matmul(
                                                ps[i][:, :], lhsT=lhsT,
                                                rhs=w[:, k,
                                                      n * 512:
                                                      (n + 1) * 512],
                                                start=(k == 0),
                                                stop=(k == KD - 1))
                                    for n in range(2):
                                        pg, pv = ps[2 * n], ps[2 * n + 1]
                                        A = m1e.tile([128, 512], F32,
                                                     tag="Asb")
                                        nc.scalar.activation(
                                            out=A[:, :], in_=pg[:, :],
                                            func=AF.Sigmoid,
                                            scale=s1a[:, m:m + 1])
                                        t1 = m1e.tile([128, 512], F32,
                                                      tag="t1sb", bufs=1)
                                        nc.vector.scalar_tensor_tensor(
                                            out=t1[:, :], in0=pg[:, :],
                                            scalar=s12a[:, m:m + 1],
                                            in1=A[:, :],
                                            op0=OP.mult, op1=OP.mult)
                                        hs_ap = hbuf[r % 2][
                                            :, ml, n * 512:(n + 1) * 512]
                                        nc.vector.tensor_mul(out=hs_ap,
                                                             in0=pv[:, :],
                                                             in1=t1[:, :])
                                        nc.vector.tensor_reduce(
                                            out=hm2[:, n:n + 1],
                                            in_=hs_ap, axis=AX.X,
                                            op=OP.max,
                                            apply_absolute_value=True)
                                    nc.vector.tensor_max(
                                        out=hmall[:, ml:ml + 1],
                                        in0=hm2[:, 0:1], in1=hm2[:, 1:2])
                                if r == 0 and qb == 2:
                                    ow_gamma_emit(rqm)
                                # out_w chunks 0..7 spread over blocks
                                # 0..4; the last lands at block 4 so the
                                # w2b data-dep pins the 32MB AllGather
                                # to ~block 4's end on the cc stream
                                owc = {(0, 3): 0, (1, 1): 1, (1, 2): 2,
                                       (2, 1): 3, (2, 2): 4, (3, 1): 5,
                                       (3, 2): 6, (4, 1): 7}.get((r, qb))
                                if owc is not None:
                                    ow_tern_batch(rqm, owc)
                                    if owc % 2 == 1:
                                        # this w2b quarter is complete:
                                        # its AllGather chunk can go (the
                                        # trigger must queue BEHIND its
                                        # producers on the GpSimd FIFO)
                                        q = owc // 2
                                        gp.collective_compute(
                                            "AllGather", OP.bypass,
                                            replica_groups=RG,
                                            ins=[w2b[q * 256:(q + 1) * 256,
                                                     :].opt()],
                                            outs=[w2gq[q][:, :].opt()])
                                if r >= 1 and qb == 3:
                                    # delayed requant of block r-1
                                    requant_emit(r - 1, rqm, h_sb(r - 1))
                            nc.sync.dma_start(
                                out=arh_i[r][0, :]
                                .rearrange("(ml p) -> p ml", p=128),
                                in_=hmall[:, :])
                            gp.collective_compute(
                                "AllReduce", OP.max, replica_groups=RG,
                                ins=[arh_i[r][:, :].opt()],
                                outs=[arh_o[r][:, :].opt()])
                        # block-7 requant, HERE inside M1 scope: reads
                        # hbuf[1] directly (no DRAM spill) and at mm1's
                        # end there is nothing left to head-of-line
                        # block. VectorE mode: M2's ScalarE/Sync carry
                        # the khT transposes.
                        requant_emit(RLAST, rqm, h_sb(RLAST),
                                     on_vector=True)

            # ================= phase M2: mm2, two r-groups =================
            with (
                tc.tile_pool(name="m2k", bufs=1) as m2k,
                tc.tile_pool(name="m2w", bufs=8) as m2w,
                tc.tile_pool(name="m2o", bufs=4) as m2o,
                tc.tile_pool(name="rq2", bufs=2) as rq2,
            ):
                khTg = [m2k.tile([128, KH, 128], BF16, tag=f"khTg{r}",
                                 name=f"khTg{r}") for r in range(N_CORES)]
                Woq = [w2gq[q][:, :].rearrange("(n p) d -> n p d", p=128)
                       for q in range(4)]

                def Wo(k):     # global hidden row-chunk k of ternary out_w
                    return Woq[(k % 8) // 2][(k // 8) * 2 + (k % 2)]

                Outv = out_d.ap().rearrange("(r p) d -> r p d", p=128)

                def khTg_fill(r):
                    """khT for block r straight from the A2A output:
                    8 transpose-DMAs (Scalar HWDGE only — concurrent
                    transposes from two queues corrupt data)."""
                    for j in range(N_CORES):
                        nc.scalar.dma_start(
                            out=khTg[r][:, j * KHL:(j + 1) * KHL, :],
                            in_=a2o[r][j], transpose=True)

                # khT transposes j-major over r0..6 so the first k-rows
                # of every block land first; r=7 behind its A2A
                for j in range(N_CORES):
                    for r in range(N_CORES - 1):
                        nc.scalar.dma_start(
                            out=khTg[r][:, j * KHL:(j + 1) * KHL, :],
                            in_=a2o[r][j], transpose=True)
                khTg_fill(RLAST)

                def evict(po_r, r, dcol):
                    ot = m2o.tile([128, 512], F32, tag="ot")
                    nc.scalar.activation(out=ot[:, :], in_=po_r[:, :],
                                         func=AF.Copy,
                                         scale=sosel[r][:, :])
                    # Sync, NOT GpSimd: an eviction ahead of the a2i(7)
                    # writes on the GpSimd FIFO would delay the AllToAll
                    # trigger behind whole matmul passes
                    nc.sync.dma_start(
                        out=Outv[r][:, dcol * 512:(dcol + 1) * 512],
                        in_=ot[:, :])

                def col_pass(grp, dcol):
                    """One dcol pass over up to 8 token-blocks: one 1KB
                    weight tile per len(grp) matmuls, weights read once
                    per dcol."""
                    po = [psp.tile([128, 512], F32, tag="ps",
                                   name=f"po{grp[0]}_{dcol}_{i}")
                          for i in range(len(grp))]
                    for k in range(KH):
                        w2ti = m2w.tile([128, 512], mybir.dt.int8,
                                        tag="w2ti")
                        nc.sync.dma_start(
                            out=w2ti[:, :],
                            in_=Wo(k)[:, dcol * 512:(dcol + 1) * 512])
                        w2t = m2w.tile([128, 512], BF16, tag="w2t")
                        nc.vector.tensor_scalar_add(out=w2t[:, :],
                                                    in0=w2ti[:, :],
                                                    scalar1=0.0)
                        for i, r in enumerate(grp):
                            nc.tensor.matmul(
                                po[i][:, :],
                                lhsT=khTg[r][:, k, :],
                                rhs=w2t[:, :],
                                start=(k == 0), stop=(k == KH - 1))
                    for i, r in enumerate(grp):
                        evict(po[i], r, dcol)

                # dcol0 without r=7 (its AllToAll is still in flight),
                # then full passes, then the small r=7/dcol0 make-up
                col_pass(list(range(7)), 0)
                for dcol in range(1, 4):
                    col_pass(list(range(8)), dcol)
                col_pass([RLAST], 0)


_NC_CACHE = {}


def _get_nc():
    if "nc" not in _NC_CACHE:
        _NC_CACHE["nc"] = _build()
    return _NC_CACHE["nc"]


def kernel(x, gate_w, gate_b, val_w, val_b, out_w, out_b, _trace=False):
    x = np.ascontiguousarray(np.asarray(x), dtype=np.float32)
    gate_w = np.asarray(gate_w, dtype=np.float32)
    val_w = np.asarray(val_w, dtype=np.float32)
    out_w = np.asarray(out_w, dtype=np.float32)
    gate_b = np.asarray(gate_b)
    val_b = np.asarray(val_b)
    out_b = np.asarray(out_b)
    assert not np.any(gate_b) and not np.any(val_b), (
        "device kernel folds silu(y+b) with b=0; nonzero gate/val bias "
        "not supported")

    orig_shape = x.shape
    xf = x.reshape(-1, x.shape[-1])
    assert xf.shape == (T, D) and gate_w.shape == (H, D)
    assert val_w.shape == (H, D) and out_w.shape == (D, H)
    xi = xf.reshape(N_CORES, N_CORES, 128, D)        # [r, i, p, d]

    nc = _get_nc()
    in_maps = []
    for i in range(N_CORES):
        sel = np.zeros((1, N_CORES), np.float32)
        sel[0, i] = 1.0
        in_maps.append({
            "x_own": np.ascontiguousarray(xi[:, i].reshape(TL, D)),
            "gwT": np.ascontiguousarray(gate_w[i * HL:(i + 1) * HL, :].T),
            "vwT": np.ascontiguousarray(val_w[i * HL:(i + 1) * HL, :].T),
            "owT": np.ascontiguousarray(out_w[:, i * HL:(i + 1) * HL].T),
            "sel8": sel,
        })
    res = run_bass_kernel_spmd(nc, in_maps, core_ids=list(range(N_CORES)),
                               trace=_trace)
    # core i owns tokens r*1024 + i*128 + [0,128) for r in 0..7
    out = np.empty((T, D), np.float32)
    ov = out.reshape(N_CORES, N_CORES, 128, D)       # [r, i, p, d]
    for i in range(N_CORES):
        ov[:, i] = res.results[i]["out"].reshape(N_CORES, 128, D)
    out = out + out_b[None, :].astype(np.float32)
    kernel._last_results = res
    return out.reshape(orig_shape)


# revision 63
# speedup vs baseline: 1.0512x; 1.0512x over previous
"""BitSwiGLU Trainium2 kernel — tensor-parallel over hidden, 8 NeuronCores.

Math (per bit_linear, forward values):
    gamma_x = clip(max|x_row|, 1e-5);  k = rne(x * 127/gamma_x)  in [-127,127]
    gamma_w = clip(mean|w|, 1e-5);    t = sign(w) * (|w| > 0.5*gamma_w)
    y = (k @ t.T) * (gamma_x*gamma_w/127) + b

k and t are small integers, exactly representable in bf16; the TensorEngine
accumulates bf16 products in fp32 PSUM, so k @ t.T is EXACT integer math at
bf16 speed. Ternarization runs as t2 = sign(w-thr)+sign(w+thr) in {-2,0,2};
the factor 2 is folded into the eviction scales.

v4 design notes (driven by the v3 trace):
 - The HAM trace showed the PE clock held at K=13/16 (1.95 GHz) through
   mm1 while M2 (PE-only) ran at 8/8 (2.4 GHz): the chip power budget
   throttles the PE when VectorE/ScalarE/DMA run hot alongside it. So
   redundant work is not free even when perfectly overlapped.
 - x-quant is therefore SHARDED: the host hands each core only its own
   1024 tokens (x_own, same interleaving as the output). Each core
   quantizes 1 of 8 token-chunks per block and the integer kxT slabs are
   AllGather-ed (0.5MB -> 4MB per block, triggered TWO blocks early so
   the collective stream has slack). gamma_x rides along as a bf16
   hi/lo pair in 2 extra columns (rel err 2^-17; only the eviction
   scales consume it — the owner core quantizes with exact f32).
   This cuts per-core mm1 DVE+ACT work ~2x and HBM reads 8x on x.
 - mm2 lhsT (khT) is assembled in M2 directly from the AllToAll output
   with transpose-DMAs into SBUF group tiles (no DRAM round trip, no
   mm1-time transposes): M2 has both SBUF room and power headroom.
 - h stays in SBUF (hbuf, 2 block buffers); only block 7 spills (its
   requant runs in M2 scope). requant_block(r) emits at block r+1 qb3
   so its gamma_h AllReduce(max) wait is satisfied before it reaches
   the strict-FIFO queues. Rounding steps run on ScalarE.
 - out_w ternarize is spread thin over blocks 0..5 (two 1MB batches per
   block); its 32MB AllGather triggers after AR(5) — the one spot where
   the single collective stream has a ~2-block quiet window.
 - mm2 runs dcol-PAIR passes over two 4-block groups: 8 PSUM banks, one
   2KB weight tile per 8 matmuls so the weight stream keeps pace.

Token ownership: core i owns global tokens {r*1024 + i*128 + [0,128) for
r in 0..7}; the host wrapper slices x_own the same way and re-interleaves
the 8 per-core outputs.
"""

import numpy as np

import concourse.mybir as mybir
import concourse.tile as tile
from concourse import bacc
from concourse import bass_isa
from concourse.bass_utils import run_bass_kernel_spmd

F32 = mybir.dt.float32
BF16 = mybir.dt.bfloat16
AF = mybir.ActivationFunctionType
OP = mybir.AluOpType
AX = mybir.AxisListType

MAGIC = 12582912.0  # 1.5 * 2**23 : (v + MAGIC) - MAGIC == rne(v) for |v| < 2**22

N_CORES = 8
D = 2048            # d_model
H = 8192            # hidden (full)
HL = H // N_CORES   # 1024 hidden per core
T = 8192            # total tokens
TL = T // N_CORES   # 1024 tokens owned per core
KD = D // 128       # 16 contraction chunks, mm1
KHL = HL // 128     # 8  chunks of the local hidden slice
KH = H // 128       # 64 contraction chunks, mm2
RLAST = N_CORES - 1
RG = [list(range(N_CORES))]
AGW = KD * 128 + 2  # kx AllGather row width: kxT flat + gamma_x hi/lo


def _build():
    nc = bacc.Bacc("TRN2", target_bir_lowering=False, debug=False,
                   num_devices=N_CORES)
    xo_d = nc.dram_tensor("x_own", [TL, D], F32, kind="ExternalInput")
    gwT_d = nc.dram_tensor("gwT", [D, HL], F32, kind="ExternalInput")
    vwT_d = nc.dram_tensor("vwT", [D, HL], F32, kind="ExternalInput")
    owT_d = nc.dram_tensor("owT", [HL, D], F32, kind="ExternalInput")
    sel_d = nc.dram_tensor("sel8", [1, N_CORES], F32, kind="ExternalInput")
    out_d = nc.dram_tensor("out", [TL, D], F32, kind="ExternalOutput")

    with tile.TileContext(nc) as tc:
        _body(tc, xo_d, gwT_d, vwT_d, owT_d, sel_d, out_d)
    nc.compile()
    return nc


def _body(tc, xo_d, gwT_d, vwT_d, owT_d, sel_d, out_d):
    nc = tc.nc
    gp = nc.gpsimd

    with (
        tc.tile_pool(name="pp", bufs=1) as pp,
        tc.tile_pool(name="psp", bufs=8, space="PSUM") as psp,
        tc.tile_pool(name="drp", bufs=1, space="DRAM") as drp,
    ):
        # ---------- DRAM scratch ----------
        gv_i = drp.tile([1, 4], F32, tag="gv_i")
        gv_o = drp.tile([1, 4], F32, tag="gv_o", addr_space="Shared")
        go_i = drp.tile([1, 4], F32, tag="go_i")
        go_o = drp.tile([1, 4], F32, tag="go_o", addr_space="Shared")
        # ternary out_w travels as INT8 ({-2,0,2} exact): halves the
        # AllGather wire AND mm2's weight-stream HBM draw; VectorE (idle
        # in M2) casts tiles back to bf16 for the PE. AG in 4 chunks of
        # 256 rows/core: chunk q holds rows i*1024 + q*256 + [0,256)
        I8 = mybir.dt.int8
        w2b = drp.tile([HL, D], I8, tag="w2b")
        w2gq = [drp.tile([N_CORES * 256, D], I8, tag=f"w2gq{q}",
                         name=f"w2gq{q}", addr_space="Shared")
                for q in range(4)]
        arh_i = [drp.tile([1, TL], F32, tag=f"arhi{r}", name=f"arhi{r}")
                 for r in range(N_CORES)]
        arh_o = [drp.tile([1, TL], F32, tag=f"arho{r}", name=f"arho{r}",
                          addr_space="Shared") for r in range(N_CORES)]
        a2i = [drp.tile([N_CORES, 128, HL], BF16, tag=f"a2i{r}",
                        name=f"a2i{r}") for r in range(N_CORES)]
        a2o = [drp.tile([N_CORES, 128, HL], BF16, tag=f"a2o{r}",
                        name=f"a2o{r}") for r in range(N_CORES)]
        agx_i = [drp.tile([128, AGW], BF16, tag=f"agxi{r}",
                          name=f"agxi{r}") for r in range(N_CORES)]
        agx_o = [drp.tile([N_CORES, 128, AGW], BF16, tag=f"agxo{r}",
                          name=f"agxo{r}", addr_space="Shared")
                 for r in range(N_CORES)]

        # ---------- persistent SBUF (whole program) ----------
        gam = pp.tile([128, 4], F32, tag="gam")             # g, v, o gammas
        thr = pp.tile([128, 6], F32, tag="thr")             # +-thr g/v/o
        gxall = pp.tile([128, 64], F32, tag="gxall")        # gamma_x per tok
        s1a = pp.tile([128, 64], F32, tag="s1a")
        s2a = pp.tile([128, 64], F32, tag="s2a")
        s12a = pp.tile([128, 64], F32, tag="s12a")
        selb = pp.tile([128, N_CORES], F32, tag="selb")
        sofull = [pp.tile([128, KHL], F32, tag=f"sofull{r}",
                          name=f"sofull{r}") for r in range(N_CORES)]
        sosel = [pp.tile([128, 1], F32, tag=f"sosel{r}", name=f"sosel{r}")
                 for r in range(N_CORES)]

        Gv = gwT_d.ap().rearrange("(c p) h -> c p h", p=128)    # 16 x [128,HL]
        Vv = vwT_d.ap().rearrange("(c p) h -> c p h", p=128)
        Ov = owT_d.ap().rearrange("(c p) d -> c p d", p=128)    # 8 x [128,D]
        Xo = xo_d.ap().rearrange("(r p) d -> r p d", p=128)     # 8 x [128,D]

        thr_g, nthr_g = thr[:, 0:1], thr[:, 1:2]
        thr_v, nthr_v = thr[:, 2:3], thr[:, 3:4]
        thr_o, nthr_o = thr[:, 4:5], thr[:, 5:6]

        def tern_act(pool, wt_ap, out_ap, thr_p, thr_n, w=HL):
            # bufs=4: the ScalarE Sign stream must not be throttled by
            # the VectorE adds' buffer returns (vector runs a backlog
            # during the prologue ternarize)
            sp = pool.tile([128, w], BF16, tag="q_sp", bufs=3)
            nc.scalar.activation(out=sp[:, :], in_=wt_ap,
                                 func=AF.Sign, bias=thr_n)
            sn = pool.tile([128, w], BF16, tag="q_sn", bufs=3)
            nc.scalar.activation(out=sn[:, :], in_=wt_ap,
                                 func=AF.Sign, bias=thr_p)
            nc.vector.tensor_add(out=out_ap, in0=sp[:, :], in1=sn[:, :])

        def tern_dve(pool, wt_ap, out_ap, thr_p, thr_n, w=HL):
            mp = pool.tile([128, w], BF16, tag="q_sp", bufs=3)
            nc.vector.tensor_scalar(out=mp[:, :], in0=wt_ap,
                                    scalar1=thr_p, scalar2=2.0,
                                    op0=OP.is_gt, op1=OP.mult)
            mn = pool.tile([128, w], BF16, tag="q_sn", bufs=3)
            nc.vector.tensor_scalar(out=mn[:, :], in0=wt_ap,
                                    scalar1=thr_n, scalar2=2.0,
                                    op0=OP.is_lt, op1=OP.mult)
            nc.vector.tensor_sub(out=out_ap, in0=mp[:, :], in1=mn[:, :])

        # rqs: tiny per-r requant scratch, lives through M2
        with tc.tile_pool(name="rqs", bufs=1) as rqs:
            ghr = [rqs.tile([128, KHL], F32, tag=f"ghr{r}", name=f"ghr{r}")
                   for r in range(N_CORES)]
            shr = [rqs.tile([128, KHL], F32, tag=f"shr{r}", name=f"shr{r}")
                   for r in range(N_CORES)]

            def requant_scales(r, pool):
                """AllReduced per-token max -> eviction + requant scales."""
                nc.sync.dma_start(
                    out=ghr[r][:, :],
                    in_=arh_o[r][0, :].rearrange("(ml p) -> p ml", p=128))
                gcl = pool.tile([128, KHL], F32, tag="gcl")
                nc.vector.tensor_scalar_max(out=gcl[:, :], in0=ghr[r][:, :],
                                            scalar1=1e-5)
                nc.vector.tensor_scalar(out=sofull[r][:, :], in0=gcl[:, :],
                                        scalar1=gam[:, 2:3],
                                        scalar2=1.0 / 254.0,
                                        op0=OP.mult, op1=OP.mult)
                solm = pool.tile([128, KHL], F32, tag="solm")
                nc.vector.tensor_mul(out=solm[:, :], in0=sofull[r][:, :],
                                     in1=selb[:, :])
                nc.vector.tensor_reduce(out=sosel[r][:, :], in_=solm[:, :],
                                        axis=AX.X, op=OP.add)
                rcph = pool.tile([128, KHL], F32, tag="rcph")
                nc.vector.reciprocal(out=rcph[:, :], in_=gcl[:, :])
                nc.vector.tensor_scalar_mul(out=shr[r][:, :], in0=rcph[:, :],
                                            scalar1=127.0)

            def requant_emit(r, pool, h_slice, on_vector=False):
                """Quantize h block r to bf16 integer levels + AllToAll.
                h_slice(ml, hf) -> [128,512] AP of h (f32). Rounding runs
                on ScalarE during mm1 (VectorE is hotter there) but on
                VectorE for the block-7 instance in M2 (ScalarE is busy
                with khT transposes there)."""
                requant_scales(r, pool)
                for ml in range(KHL):
                    for hf in range(2):
                        hmg = pool.tile([128, HL // 2], F32, tag="hmg")
                        kh = pool.tile([128, HL // 2], BF16, tag="kh")
                        if on_vector:
                            # split across engines: the hbuf reads (hmg)
                            # finish in half the time, releasing M2's
                            # khT-transpose WAR earlier
                            nc.scalar.activation(
                                out=hmg[:, :], in_=h_slice(ml, hf),
                                func=AF.Copy,
                                scale=shr[r][:, ml:ml + 1], bias=MAGIC)
                            nc.vector.tensor_scalar_sub(
                                out=kh[:, :], in0=hmg[:, :], scalar1=MAGIC)
                        else:
                            nc.scalar.activation(
                                out=hmg[:, :], in_=h_slice(ml, hf),
                                func=AF.Copy,
                                scale=shr[r][:, ml:ml + 1], bias=MAGIC)
                            nc.scalar.activation(
                                out=kh[:, :], in_=hmg[:, :],
                                func=AF.Copy, bias=-MAGIC)
                        gp.dma_start(
                            out=a2i[r][ml, :,
                                       hf * (HL // 2):(hf + 1) * (HL // 2)],
                            in_=kh[:, :])
                gp.collective_compute("AllToAll", OP.bypass,
                                      replica_groups=RG,
                                      ins=[a2i[r][:, :, :].opt()],
                                      outs=[a2o[r][:, :, :].opt()])

            # =============== prologue ===============
            with (
                tc.tile_pool(name="wW", bufs=1) as wW,
                tc.tile_pool(name="kxp", bufs=2) as kxp,
                tc.tile_pool(name="xq", bufs=2) as xq,
            ):
                def quant_own(r):
                    """Quantize this core's 128-token chunk of block r,
                    pack kxT + gamma_x(hi/lo bf16) into agx_i[r]."""
                    xt = xq.tile([128, D], F32, tag="x_in")
                    nc.sync.dma_start(out=xt[:, :], in_=Xo[r])
                    gxo = xq.tile([128, 1], F32, tag="gxo")
                    gmx = xq.tile([128, 1], F32, tag="gmx")
                    nc.vector.tensor_reduce(out=gmx[:, :], in_=xt[:, :],
                                            axis=AX.X, op=OP.max,
                                            apply_absolute_value=True)
                    nc.vector.tensor_scalar_max(out=gxo[:, :],
                                                in0=gmx[:, :], scalar1=1e-5)
                    rcp = xq.tile([128, 1], F32, tag="rcpx")
                    nc.vector.reciprocal(out=rcp[:, :], in_=gxo[:, :])
                    sx = xq.tile([128, 1], F32, tag="sx")
                    nc.vector.tensor_scalar_mul(out=sx[:, :], in0=rcp[:, :],
                                                scalar1=127.0)
                    nc.vector.tensor_scalar(out=xt[:, :], in0=xt[:, :],
                                            scalar1=sx[:, :], scalar2=MAGIC,
                                            op0=OP.mult, op1=OP.add)
                    kx = xq.tile([128, D], BF16, tag="kx", bufs=1)
                    nc.scalar.activation(out=kx[:, :], in_=xt[:, :],
                                         func=AF.Copy, bias=-MAGIC)
                    kxT = xq.tile([128, KD, 128], BF16, tag="kxT")
                    nc.scalar.dma_start(out=kxT[:, :, :], in_=kx[:, :],
                                        transpose=True)
                    sc2 = xq.tile([128, 2], BF16, tag="sc2")
                    nc.vector.tensor_scalar_add(out=sc2[:, 0:1],
                                                in0=gxo[:, :], scalar1=0.0)
                    nc.vector.tensor_sub(out=sc2[:, 1:2], in0=gxo[:, :],
                                         in1=sc2[:, 0:1])
                    gp.dma_start(
                        out=agx_i[r][:, 0:KD * 128],
                        in_=kxT[:, :, :].rearrange("p k t -> p (k t)"))
                    gp.dma_start(out=agx_i[r][:, KD * 128:AGW],
                                 in_=sc2[:, :])
                    gp.collective_compute("AllGather", OP.bypass,
                                          replica_groups=RG,
                                          ins=[agx_i[r][:, :].opt()],
                                          outs=[agx_o[r][:, :, :].opt()])

                if True:
                    # per-chunk tiles: the tern(c) -> matmul(k=c)
                    # dependency stays fine-grained, so block 0's
                    # matmuls stream behind the ternarize instead of
                    # waiting for the whole tensor
                    WgT = [wW.tile([128, HL], BF16, tag=f"WgT{c}",
                                   name=f"WgT{c}") for c in range(KD)]
                    WvT = [wW.tile([128, HL], BF16, tag=f"WvT{c}",
                                   name=f"WvT{c}") for c in range(KD)]
                    with tc.tile_pool(name="stg", bufs=2) as stg:
                        Gst = stg.tile([128, KD, HL], F32, tag="Gst",
                                       bufs=1)                      # 8.4 MB
                        parts = stg.tile([128, 4 * KD], F32, tag="parts",
                                         bufs=1)
                        # gate: load + stage + |w|-accum on ScalarE;
                        # val: rotating load + |w|-reduce on VectorE
                        for c in range(KD):
                            nc.sync.dma_start(out=Gst[:, c, :], in_=Gv[c])
                            scr = stg.tile([128, HL], F32, tag="scr", bufs=1)
                            nc.scalar.activation(
                                out=scr[:, :], in_=Gst[:, c, :],
                                func=AF.Abs,
                                accum_out=parts[:, c:c + 1])
                            wt = stg.tile([128, HL], F32, tag="v_in",
                                          bufs=4)
                            nc.sync.dma_start(out=wt[:, :], in_=Vv[c])
                            nc.vector.tensor_reduce(
                                out=parts[:, KD + c:KD + c + 1],
                                in_=wt[:, :], axis=AX.X, op=OP.add,
                                apply_absolute_value=True)
                        gsum = stg.tile([128, 4], F32, tag="gsum", bufs=1)
                        nc.vector.memset(gsum[:, :], 0.0)
                        for j, sl in enumerate((slice(0, KD),
                                                slice(KD, 2 * KD))):
                            red = stg.tile([128, 1], F32, tag="red")
                            nc.vector.tensor_reduce(out=red[:, :],
                                                    in_=parts[:, sl],
                                                    axis=AX.X, op=OP.add)
                            gp.partition_all_reduce(gsum[:, j:j + 1],
                                                    red[:, :], 128,
                                                    bass_isa.ReduceOp.add)
                        nc.sync.dma_start(out=gv_i[0:1, :],
                                          in_=gsum[0:1, :])
                        gp.collective_compute("AllReduce", OP.add,
                                              replica_groups=RG,
                                              ins=[gv_i[:, :].opt()],
                                              outs=[gv_o[:, :].opt()])
                        # own-token quant for blocks 0,1 + their kx
                        # AllGathers ride right behind AllReduce #1
                        quant_own(0)
                        quant_own(1)
                        # out_w |w|-sum pass rides under AllReduce #1
                        for c in range(KHL):
                            for hf in range(2):
                                wt = stg.tile([128, HL], F32, tag="v_in",
                                              bufs=4)
                                nc.sync.dma_start(
                                    out=wt[:, :],
                                    in_=Ov[c][:, hf * HL:(hf + 1) * HL])
                                col = 2 * KD + 2 * c + hf
                                if hf == 0:
                                    scr = stg.tile([128, HL], F32,
                                                   tag="scr", bufs=1)
                                    nc.scalar.activation(
                                        out=scr[:, :], in_=wt[:, :],
                                        func=AF.Abs,
                                        accum_out=parts[:, col:col + 1])
                                else:
                                    nc.vector.tensor_reduce(
                                        out=parts[:, col:col + 1],
                                        in_=wt[:, :], axis=AX.X, op=OP.add,
                                        apply_absolute_value=True)
                        redo = stg.tile([128, 1], F32, tag="red")
                        nc.vector.tensor_reduce(
                            out=redo[:, :],
                            in_=parts[:, 2 * KD:2 * KD + 2 * KHL],
                            axis=AX.X, op=OP.add)
                        gp.partition_all_reduce(gsum[:, 2:3], redo[:, :],
                                                128, bass_isa.ReduceOp.add)
                        nc.sync.dma_start(out=go_i[0:1, :],
                                          in_=gsum[0:1, :])
                        gp.collective_compute("AllReduce", OP.add,
                                              replica_groups=RG,
                                              ins=[go_i[:, :].opt()],
                                              outs=[go_o[:, :].opt()])
                        # gammas g/v from AllReduce #1
                        g0 = stg.tile([1, 4], F32, tag="g0", bufs=1)
                        nc.sync.dma_start(out=g0[:, :], in_=gv_o[0:1, :])
                        gbc = stg.tile([128, 4], F32, tag="gbc", bufs=1)
                        gp.partition_broadcast(gbc[:, :], g0[:, :])
                        nc.vector.tensor_scalar(out=gam[:, 0:2],
                                                in0=gbc[:, 0:2],
                                                scalar1=1.0 / (H * D),
                                                scalar2=1e-5,
                                                op0=OP.mult, op1=OP.max)
                        for j in range(2):
                            nc.vector.tensor_scalar_mul(
                                out=thr[:, 2 * j:2 * j + 1],
                                in0=gam[:, j:j + 1], scalar1=0.5)
                            nc.vector.tensor_scalar_mul(
                                out=thr[:, 2 * j + 1:2 * j + 2],
                                in0=gam[:, j:j + 1], scalar1=-0.5)
                        s0 = stg.tile([1, N_CORES], F32, tag="s0", bufs=1)
                        nc.sync.dma_start(out=s0[:, :], in_=sel_d.ap())
                        gp.partition_broadcast(selb[:, :], s0[:, :])
                        # ternarize: gate from SBUF (ScalarE), val re-read
                        # (GpSimd queue) + ternarize on VectorE
                        for c in range(KD):
                            tern_act(stg, Gst[:, c, :], WgT[c][:, :],
                                     thr_g, nthr_g)
                            wtv = stg.tile([128, HL], F32, tag="v_in",
                                           bufs=4)
                            nc.sync.dma_start(out=wtv[:, :], in_=Vv[c])
                            tern_dve(stg, wtv[:, :], WvT[c][:, :],
                                     thr_v, nthr_v)

                    def ow_gamma_emit(pool):
                        """gamma_o + thresholds from AllReduce #2."""
                        g1 = pool.tile([1, 4], F32, tag="g1", bufs=1)
                        nc.sync.dma_start(out=g1[:, :], in_=go_o[0:1, :])
                        gb1 = pool.tile([128, 4], F32, tag="gb1", bufs=1)
                        gp.partition_broadcast(gb1[:, :], g1[:, :])
                        nc.vector.tensor_scalar(out=gam[:, 2:3],
                                                in0=gb1[:, 2:3],
                                                scalar1=1.0 / (H * D),
                                                scalar2=1e-5,
                                                op0=OP.mult, op1=OP.max)
                        nc.vector.tensor_scalar_mul(out=thr[:, 4:5],
                                                    in0=gam[:, 2:3],
                                                    scalar1=0.5)
                        nc.vector.tensor_scalar_mul(out=thr[:, 5:6],
                                                    in0=gam[:, 2:3],
                                                    scalar1=-0.5)

                    def ow_tern_batch(pool, c):
                        """Ternarize one of out_w's 8 row-chunks (spread
                        across mm1 blocks, two batches per block)."""
                        wts = []
                        for qf in range(4):
                            wt = pool.tile([128, 512], F32, tag="ow_in",
                                           bufs=4)
                            gp.dma_start(
                                out=wt[:, :],
                                in_=Ov[c][:, qf * 512:(qf + 1) * 512])
                            wts.append(wt)
                        for qf in range(4):
                            tq = pool.tile([128, 512], mybir.dt.int8,
                                           tag="ow_tq")
                            tern = tern_act if qf % 2 == 0 else tern_dve
                            tern(pool, wts[qf][:, :], tq[:, :], thr_o,
                                 nthr_o, w=512)
                            gp.dma_start(
                                out=w2b[c * 128:(c + 1) * 128,
                                        qf * 512:(qf + 1) * 512],
                                in_=tq[:, :])

                    # ===== phase M1: mm1 + silu + requant + A2A =====
                    with (
                        tc.tile_pool(name="m1e", bufs=2) as m1e,
                        tc.tile_pool(name="hbp", bufs=1) as hbp,
                        tc.tile_pool(name="rqm", bufs=2) as rqm,
                    ):
                        hbuf = [hbp.tile([128, KHL, HL], F32,
                                         tag=f"hbuf{b}", name=f"hbuf{b}")
                                for b in range(2)]

                        def h_sb(r):
                            def sl(ml, hf):
                                return hbuf[r % 2][:, ml,
                                                   hf * 512:(hf + 1) * 512]
                            return sl

                        for r in range(N_CORES):
                            hmall = m1e.tile([128, KHL], F32, tag="hmall",
                                             bufs=2, name=f"hmall{r}")
                            for qb in range(4):     # 256-token quarters
                                kxq = kxp.tile([128, KD, 256], BF16,
                                               tag="kxq")
                                for j in range(2):
                                    jj = qb * 2 + j
                                    nc.sync.dma_start(
                                        out=kxq[:, :,
                                                j * 128:(j + 1) * 128],
                                        in_=agx_o[r][jj, :, 0:KD * 128]
                                        .rearrange("p (k t) -> p k t",
                                                   t=128))
                                if qb == 0:
                                    if r + 2 < N_CORES:
                                        quant_own(r + 2)
                                    # reconstruct gamma_x + eviction
                                    # scales for the whole block
                                    gxsc = xq.tile([128, KHL, 2], BF16,
                                                   tag="gxsc")
                                    nc.sync.dma_start(
                                        out=gxsc[:, :, :],
                                        in_=agx_o[r][:, :, KD * 128:AGW]
                                        .rearrange("j p c -> p j c"))
                                    r8 = r * 8
                                    nc.vector.tensor_add(
                                        out=gxall[:, r8:r8 + 8],
                                        in0=gxsc[:, :, 0],
                                        in1=gxsc[:, :, 1])
                                    nc.vector.tensor_scalar(
                                        out=s1a[:, r8:r8 + 8],
                                        in0=gxall[:, r8:r8 + 8],
                                        scalar1=gam[:, 0:1],
                                        scalar2=1.0 / 254.0,
                                        op0=OP.mult, op1=OP.mult)
                                    nc.vector.tensor_scalar(
                                        out=s2a[:, r8:r8 + 8],
                                        in0=gxall[:, r8:r8 + 8],
                                        scalar1=gam[:, 1:2],
                                        scalar2=1.0 / 254.0,
                                        op0=OP.mult, op1=OP.mult)
                                    nc.vector.tensor_mul(
                                        out=s12a[:, r8:r8 + 8],
                                        in0=s1a[:, r8:r8 + 8],
                                        in1=s2a[:, r8:r8 + 8])
                                for j in range(2):
                                    ml = qb * 2 + j
                                    m = r * 8 + ml
                                    hm2 = m1e.tile([128, 2], F32,
                                                   tag="hm2")
                                    ps = [psp.tile([128, 512], F32,
                                                   tag="ps",
                                                   name=f"ps{m}_{i}")
                                          for i in range(4)]
                                    for k in range(KD):
                                        lhsT = kxq[:, k,
                                                   j * 128:(j + 1) * 128]
                                        for i, (w, n) in enumerate(
                                                ((WgT, 0), (WvT, 0),
                                                 (WgT, 1), (WvT, 1))):
                                            nc.tensor.matmul(
                                                ps[i][:, :], lhsT=lhsT,
                                                rhs=w[k][:,
                                                      n * 512:
                                                      (n + 1) * 512],
                                                start=(k == 0),
                                                stop=(k == KD - 1))
                                    for n in range(2):
                                        pg, pv = ps[2 * n], ps[2 * n + 1]
                                        A = m1e.tile([128, 512], F32,
                                                     tag="Asb")
                                        nc.scalar.activation(
                                            out=A[:, :], in_=pg[:, :],
                                            func=AF.Sigmoid,
                                            scale=s1a[:, m:m + 1])
                                        t1 = m1e.tile([128, 512], F32,
                                                      tag="t1sb", bufs=1)
                                        nc.vector.scalar_tensor_tensor(
                                            out=t1[:, :], in0=pg[:, :],
                                            scalar=s12a[:, m:m + 1],
                                            in1=A[:, :],
                                            op0=OP.mult, op1=OP.mult)
                                        hs_ap = hbuf[r % 2][
                                            :, ml, n * 512:(n + 1) * 512]
                                        nc.vector.tensor_mul(out=hs_ap,
                                                             in0=pv[:, :],
                                                             in1=t1[:, :])
                                        nc.vector.tensor_reduce(
                                            out=hm2[:, n:n + 1],
                                            in_=hs_ap, axis=AX.X,
                                            op=OP.max,
                                            apply_absolute_value=True)
                                    nc.vector.tensor_max(
                                        out=hmall[:, ml:ml + 1],
                                        in0=hm2[:, 0:1], in1=hm2[:, 1:2])
                                if r == 0 and qb == 2:
                                    ow_gamma_emit(rqm)
                                # out_w chunks 0..7 spread over blocks
                                # 0..4; the last lands at block 4 so the
                                # w2b data-dep pins the 32MB AllGather
                                # to ~block 4's end on the cc stream
                                owc = {(0, 3): 0, (1, 1): 1, (1, 2): 2,
                                       (2, 1): 3, (2, 2): 4, (3, 1): 5,
                                       (3, 2): 6, (4, 1): 7}.get((r, qb))
                                if owc is not None:
                                    ow_tern_batch(rqm, owc)
                                    if owc % 2 == 1:
                                        # this w2b quarter is complete:
                                        # its AllGather chunk can go (the
                                        # trigger must queue BEHIND its
                                        # producers on the GpSimd FIFO)
                                        q = owc // 2
                                        gp.collective_compute(
                                            "AllGather", OP.bypass,
                                            replica_groups=RG,
                                            ins=[w2b[q * 256:(q + 1) * 256,
                                                     :].opt()],
                                            outs=[w2gq[q][:, :].opt()])
                                if r >= 1 and qb == 3:
                                    # delayed requant of block r-1
                                    requant_emit(r - 1, rqm, h_sb(r - 1))
                            nc.sync.dma_start(
                                out=arh_i[r][0, :]
                                .rearrange("(ml p) -> p ml", p=128),
                                in_=hmall[:, :])
                            gp.collective_compute(
                                "AllReduce", OP.max, replica_groups=RG,
                                ins=[arh_i[r][:, :].opt()],
                                outs=[arh_o[r][:, :].opt()])
                        # block-7 requant, HERE inside M1 scope: reads
                        # hbuf[1] directly (no DRAM spill) and at mm1's
                        # end there is nothing left to head-of-line
                        # block. VectorE mode: M2's ScalarE/Sync carry
                        # the khT transposes.
                        requant_emit(RLAST, rqm, h_sb(RLAST),
                                     on_vector=True)

            # ================= phase M2: mm2, two r-groups =================
            with (
                tc.tile_pool(name="m2k", bufs=1) as m2k,
                tc.tile_pool(name="m2w", bufs=8) as m2w,
                tc.tile_pool(name="m2o", bufs=4) as m2o,
                tc.tile_pool(name="rq2", bufs=2) as rq2,
            ):
                khTg = [m2k.tile([128, KH, 128], BF16, tag=f"khTg{r}",
                                 name=f"khTg{r}") for r in range(N_CORES)]
                Woq = [w2gq[q][:, :].rearrange("(n p) d -> n p d", p=128)
                       for q in range(4)]

                def Wo(k):     # global hidden row-chunk k of ternary out_w
                    return Woq[(k % 8) // 2][(k // 8) * 2 + (k % 2)]

                Outv = out_d.ap().rearrange("(r p) d -> r p d", p=128)

                def khTg_fill(r):
                    """khT for block r straight from the A2A output:
                    8 transpose-DMAs (Scalar HWDGE only — concurrent
                    transposes from two queues corrupt data)."""
                    for j in range(N_CORES):
                        nc.scalar.dma_start(
                            out=khTg[r][:, j * KHL:(j + 1) * KHL, :],
                            in_=a2o[r][j], transpose=True)

                # khT transposes j-major over r0..6 so the first k-rows
                # of every block land first; r=7 behind its A2A
                for j in range(N_CORES):
                    for r in range(N_CORES - 1):
                        nc.scalar.dma_start(
                            out=khTg[r][:, j * KHL:(j + 1) * KHL, :],
                            in_=a2o[r][j], transpose=True)
                khTg_fill(RLAST)

                def evict(po_r, r, dcol):
                    ot = m2o.tile([128, 512], F32, tag="ot")
                    nc.scalar.activation(out=ot[:, :], in_=po_r[:, :],
                                         func=AF.Copy,
                                         scale=sosel[r][:, :])
                    # Sync, NOT GpSimd: an eviction ahead of the a2i(7)
                    # writes on the GpSimd FIFO would delay the AllToAll
                    # trigger behind whole matmul passes
                    nc.sync.dma_start(
                        out=Outv[r][:, dcol * 512:(dcol + 1) * 512],
                        in_=ot[:, :])

                def col_pass(grp, dcol):
                    """One dcol pass over up to 8 token-blocks: one 1KB
                    weight tile per len(grp) matmuls, weights read once
                    per dcol."""
                    po = [psp.tile([128, 512], F32, tag="ps",
                                   name=f"po{grp[0]}_{dcol}_{i}")
                          for i in range(len(grp))]
                    for k in range(KH):
                        w2ti = m2w.tile([128, 512], mybir.dt.int8,
                                        tag="w2ti")
                        nc.sync.dma_start(
                            out=w2ti[:, :],
                            in_=Wo(k)[:, dcol * 512:(dcol + 1) * 512])
                        w2t = m2w.tile([128, 512], BF16, tag="w2t")
                        nc.vector.tensor_scalar_add(out=w2t[:, :],
                                                    in0=w2ti[:, :],
                                                    scalar1=0.0)
                        for i, r in enumerate(grp):
                            nc.tensor.matmul(
                                po[i][:, :],
                                lhsT=khTg[r][:, k, :],
                                rhs=w2t[:, :],
                                start=(k == 0), stop=(k == KH - 1))
                    for i, r in enumerate(grp):
                        evict(po[i], r, dcol)

                # dcol0 without r=7 (its AllToAll is still in flight),
                # then full passes, then the small r=7/dcol0 make-up
                col_pass(list(range(7)), 0)
                for dcol in range(1, 4):
                    col_pass(list(range(8)), dcol)
                col_pass([RLAST], 0)


_NC_CACHE = {}


def _get_nc():
    if "nc" not in _NC_CACHE:
        _NC_CACHE["nc"] = _build()
    return _NC_CACHE["nc"]


def kernel(x, gate_w, gate_b, val_w, val_b, out_w, out_b, _trace=False):
    x = np.ascontiguousarray(np.asarray(x), dtype=np.float32)
    gate_w = np.asarray(gate_w, dtype=np.float32)
    val_w = np.asarray(val_w, dtype=np.float32)
    out_w = np.asarray(out_w, dtype=np.float32)
    gate_b = np.asarray(gate_b)
    val_b = np.asarray(val_b)
    out_b = np.asarray(out_b)
    assert not np.any(gate_b) and not np.any(val_b), (
        "device kernel folds silu(y+b) with b=0; nonzero gate/val bias "
        "not supported")

    orig_shape = x.shape
    xf = x.reshape(-1, x.shape[-1])
    assert xf.shape == (T, D) and gate_w.shape == (H, D)
    assert val_w.shape == (H, D) and out_w.shape == (D, H)
    xi = xf.reshape(N_CORES, N_CORES, 128, D)        # [r, i, p, d]

    nc = _get_nc()
    in_maps = []
    for i in range(N_CORES):
        sel = np.zeros((1, N_CORES), np.float32)
        sel[0, i] = 1.0
        in_maps.append({
            "x_own": np.ascontiguousarray(xi[:, i].reshape(TL, D)),
            "gwT": np.ascontiguousarray(gate_w[i * HL:(i + 1) * HL, :].T),
            "vwT": np.ascontiguousarray(val_w[i * HL:(i + 1) * HL, :].T),
            "owT": np.ascontiguousarray(out_w[:, i * HL:(i + 1) * HL].T),
            "sel8": sel,
        })
    res = run_bass_kernel_spmd(nc, in_maps, core_ids=list(range(N_CORES)),
                               trace=_trace)
    # core i owns tokens r*1024 + i*128 + [0,128) for r in 0..7
    out = np.empty((T, D), np.float32)
    ov = out.reshape(N_CORES, N_CORES, 128, D)       # [r, i, p, d]
    for i in range(N_CORES):
        ov[:, i] = res.results[i]["out"].reshape(N_CORES, 128, D)
    out = out + out_b[None, :].astype(np.float32)
    kernel._last_results = res
    return out.reshape(orig_shape)


# revision 66
# speedup vs baseline: 1.0576x; 1.0061x over previous
"""BitSwiGLU Trainium2 kernel — tensor-parallel over hidden, 8 NeuronCores.

Math (per bit_linear, forward values):
    gamma_x = clip(max|x_row|, 1e-5);  k = rne(x * 127/gamma_x)  in [-127,127]
    gamma_w = clip(mean|w|, 1e-5);    t = sign(w) * (|w| > 0.5*gamma_w)
    y = (k @ t.T) * (gamma_x*gamma_w/127) + b

k and t are small integers, exactly representable in bf16; the TensorEngine
accumulates bf16 products in fp32 PSUM, so k @ t.T is EXACT integer math at
bf16 speed. Ternarization runs as t2 = sign(w-thr)+sign(w+thr) in {-2,0,2};
the factor 2 is folded into the eviction scales.

v4 design notes (driven by the v3 trace):
 - The HAM trace showed the PE clock held at K=13/16 (1.95 GHz) through
   mm1 while M2 (PE-only) ran at 8/8 (2.4 GHz): the chip power budget
   throttles the PE when VectorE/ScalarE/DMA run hot alongside it. So
   redundant work is not free even when perfectly overlapped.
 - x-quant is therefore SHARDED: the host hands each core only its own
   1024 tokens (x_own, same interleaving as the output). Each core
   quantizes 1 of 8 token-chunks per block and the integer kxT slabs are
   AllGather-ed (0.5MB -> 4MB per block, triggered TWO blocks early so
   the collective stream has slack). gamma_x rides along as a bf16
   hi/lo pair in 2 extra columns (rel err 2^-17; only the eviction
   scales consume it — the owner core quantizes with exact f32).
   This cuts per-core mm1 DVE+ACT work ~2x and HBM reads 8x on x.
 - mm2 lhsT (khT) is assembled in M2 directly from the AllToAll output
   with transpose-DMAs into SBUF group tiles (no DRAM round trip, no
   mm1-time transposes): M2 has both SBUF room and power headroom.
 - h stays in SBUF (hbuf, 2 block buffers); only block 7 spills (its
   requant runs in M2 scope). requant_block(r) emits at block r+1 qb3
   so its gamma_h AllReduce(max) wait is satisfied before it reaches
   the strict-FIFO queues. Rounding steps run on ScalarE.
 - out_w ternarize is spread thin over blocks 0..5 (two 1MB batches per
   block); its 32MB AllGather triggers after AR(5) — the one spot where
   the single collective stream has a ~2-block quiet window.
 - mm2 runs dcol-PAIR passes over two 4-block groups: 8 PSUM banks, one
   2KB weight tile per 8 matmuls so the weight stream keeps pace.

Token ownership: core i owns global tokens {r*1024 + i*128 + [0,128) for
r in 0..7}; the host wrapper slices x_own the same way and re-interleaves
the 8 per-core outputs.
"""

import numpy as np

import concourse.mybir as mybir
import concourse.tile as tile
from concourse import bacc
from concourse import bass_isa
from concourse.bass_utils import run_bass_kernel_spmd

F32 = mybir.dt.float32
BF16 = mybir.dt.bfloat16
AF = mybir.ActivationFunctionType
OP = mybir.AluOpType
AX = mybir.AxisListType

MAGIC = 12582912.0  # 1.5 * 2**23 : (v + MAGIC) - MAGIC == rne(v) for |v| < 2**22

N_CORES = 8
D = 2048            # d_model
H = 8192            # hidden (full)
HL = H // N_CORES   # 1024 hidden per core
T = 8192            # total tokens
TL = T // N_CORES   # 1024 tokens owned per core
KD = D // 128       # 16 contraction chunks, mm1
KHL = HL // 128     # 8  chunks of the local hidden slice
KH = H // 128       # 64 contraction chunks, mm2
RLAST = N_CORES - 1
RG = [list(range(N_CORES))]
AGW = KD * 128 + 2  # kx AllGather row width: kxT flat + gamma_x hi/lo


def _build():
    nc = bacc.Bacc("TRN2", target_bir_lowering=False, debug=False,
                   num_devices=N_CORES)
    xo_d = nc.dram_tensor("x_own", [TL, D], F32, kind="ExternalInput")
    gwT_d = nc.dram_tensor("gwT", [D, HL], F32, kind="ExternalInput")
    vwT_d = nc.dram_tensor("vwT", [D, HL], F32, kind="ExternalInput")
    owT_d = nc.dram_tensor("owT", [HL, D], F32, kind="ExternalInput")
    sel_d = nc.dram_tensor("sel8", [1, N_CORES], F32, kind="ExternalInput")
    out_d = nc.dram_tensor("out", [TL, D], F32, kind="ExternalOutput")

    with tile.TileContext(nc) as tc:
        _body(tc, xo_d, gwT_d, vwT_d, owT_d, sel_d, out_d)
    nc.compile()
    return nc


def _body(tc, xo_d, gwT_d, vwT_d, owT_d, sel_d, out_d):
    nc = tc.nc
    gp = nc.gpsimd

    with (
        tc.tile_pool(name="pp", bufs=1) as pp,
        tc.tile_pool(name="psp", bufs=8, space="PSUM") as psp,
        tc.tile_pool(name="drp", bufs=1, space="DRAM") as drp,
    ):
        # ---------- DRAM scratch ----------
        gv_i = drp.tile([1, 4], F32, tag="gv_i")
        gv_o = drp.tile([1, 4], F32, tag="gv_o", addr_space="Shared")
        go_i = drp.tile([1, 4], F32, tag="go_i")
        go_o = drp.tile([1, 4], F32, tag="go_o", addr_space="Shared")
        # ternary out_w travels as INT8 ({-2,0,2} exact): halves the
        # AllGather wire AND mm2's weight-stream HBM draw; VectorE (idle
        # in M2) casts tiles back to bf16 for the PE. AG in 4 chunks of
        # 256 rows/core: chunk q holds rows i*1024 + q*256 + [0,256)
        I8 = mybir.dt.int8
        w2b = drp.tile([HL, D], I8, tag="w2b")
        w2gq = [drp.tile([N_CORES * 256, D], I8, tag=f"w2gq{q}",
                         name=f"w2gq{q}", addr_space="Shared")
                for q in range(4)]
        arh_i = [drp.tile([1, TL], F32, tag=f"arhi{r}", name=f"arhi{r}")
                 for r in range(N_CORES)]
        arh_o = [drp.tile([1, TL], F32, tag=f"arho{r}", name=f"arho{r}",
                          addr_space="Shared") for r in range(N_CORES)]
        a2i = [drp.tile([N_CORES, 128, HL], BF16, tag=f"a2i{r}",
                        name=f"a2i{r}") for r in range(N_CORES)]
        a2o = [drp.tile([N_CORES, 128, HL], BF16, tag=f"a2o{r}",
                        name=f"a2o{r}") for r in range(N_CORES)]
        agx_i = [drp.tile([128, AGW], BF16, tag=f"agxi{r}",
                          name=f"agxi{r}") for r in range(N_CORES)]
        agx_o = [drp.tile([N_CORES, 128, AGW], BF16, tag=f"agxo{r}",
                          name=f"agxo{r}", addr_space="Shared")
                 for r in range(N_CORES)]

        # ---------- persistent SBUF (whole program) ----------
        gam = pp.tile([128, 4], F32, tag="gam")             # g, v, o gammas
        thr = pp.tile([128, 6], F32, tag="thr")             # +-thr g/v/o
        gxall = pp.tile([128, 64], F32, tag="gxall")        # gamma_x per tok
        s1a = pp.tile([128, 64], F32, tag="s1a")
        s2a = pp.tile([128, 64], F32, tag="s2a")
        s12a = pp.tile([128, 64], F32, tag="s12a")
        selb = pp.tile([128, N_CORES], F32, tag="selb")
        sofull = [pp.tile([128, KHL], F32, tag=f"sofull{r}",
                          name=f"sofull{r}") for r in range(N_CORES)]
        sosel = [pp.tile([128, 1], F32, tag=f"sosel{r}", name=f"sosel{r}")
                 for r in range(N_CORES)]

        Gv = gwT_d.ap().rearrange("(c p) h -> c p h", p=128)    # 16 x [128,HL]
        Vv = vwT_d.ap().rearrange("(c p) h -> c p h", p=128)
        Ov = owT_d.ap().rearrange("(c p) d -> c p d", p=128)    # 8 x [128,D]
        Xo = xo_d.ap().rearrange("(r p) d -> r p d", p=128)     # 8 x [128,D]

        thr_g, nthr_g = thr[:, 0:1], thr[:, 1:2]
        thr_v, nthr_v = thr[:, 2:3], thr[:, 3:4]
        thr_o, nthr_o = thr[:, 4:5], thr[:, 5:6]

        def tern_act(pool, wt_ap, out_ap, thr_p, thr_n, w=HL):
            # bufs=4: the ScalarE Sign stream must not be throttled by
            # the VectorE adds' buffer returns (vector runs a backlog
            # during the prologue ternarize)
            sp = pool.tile([128, w], BF16, tag="q_sp", bufs=3)
            nc.scalar.activation(out=sp[:, :], in_=wt_ap,
                                 func=AF.Sign, bias=thr_n)
            sn = pool.tile([128, w], BF16, tag="q_sn", bufs=3)
            nc.scalar.activation(out=sn[:, :], in_=wt_ap,
                                 func=AF.Sign, bias=thr_p)
            nc.vector.tensor_add(out=out_ap, in0=sp[:, :], in1=sn[:, :])

        def tern_dve(pool, wt_ap, out_ap, thr_p, thr_n, w=HL):
            mp = pool.tile([128, w], BF16, tag="q_sp", bufs=3)
            nc.vector.tensor_scalar(out=mp[:, :], in0=wt_ap,
                                    scalar1=thr_p, scalar2=2.0,
                                    op0=OP.is_gt, op1=OP.mult)
            mn = pool.tile([128, w], BF16, tag="q_sn", bufs=3)
            nc.vector.tensor_scalar(out=mn[:, :], in0=wt_ap,
                                    scalar1=thr_n, scalar2=2.0,
                                    op0=OP.is_lt, op1=OP.mult)
            nc.vector.tensor_sub(out=out_ap, in0=mp[:, :], in1=mn[:, :])

        # rqs: tiny per-r requant scratch, lives through M2
        with tc.tile_pool(name="rqs", bufs=1) as rqs:
            ghr = [rqs.tile([128, KHL], F32, tag=f"ghr{r}", name=f"ghr{r}")
                   for r in range(N_CORES)]
            shr = [rqs.tile([128, KHL], F32, tag=f"shr{r}", name=f"shr{r}")
                   for r in range(N_CORES)]

            def requant_scales(r, pool):
                """AllReduced per-token max -> eviction + requant scales."""
                nc.sync.dma_start(
                    out=ghr[r][:, :],
                    in_=arh_o[r][0, :].rearrange("(ml p) -> p ml", p=128))
                gcl = pool.tile([128, KHL], F32, tag="gcl")
                nc.vector.tensor_scalar_max(out=gcl[:, :], in0=ghr[r][:, :],
                                            scalar1=1e-5)
                nc.vector.tensor_scalar(out=sofull[r][:, :], in0=gcl[:, :],
                                        scalar1=gam[:, 2:3],
                                        scalar2=1.0 / 254.0,
                                        op0=OP.mult, op1=OP.mult)
                solm = pool.tile([128, KHL], F32, tag="solm")
                nc.vector.tensor_mul(out=solm[:, :], in0=sofull[r][:, :],
                                     in1=selb[:, :])
                nc.vector.tensor_reduce(out=sosel[r][:, :], in_=solm[:, :],
                                        axis=AX.X, op=OP.add)
                rcph = pool.tile([128, KHL], F32, tag="rcph")
                nc.vector.reciprocal(out=rcph[:, :], in_=gcl[:, :])
                nc.vector.tensor_scalar_mul(out=shr[r][:, :], in0=rcph[:, :],
                                            scalar1=127.0)

            def requant_emit(r, pool, h_slice, on_vector=False):
                """Quantize h block r to bf16 integer levels + AllToAll.
                h_slice(ml, hf) -> [128,512] AP of h (f32). Rounding runs
                on ScalarE during mm1 (VectorE is hotter there) but on
                VectorE for the block-7 instance in M2 (ScalarE is busy
                with khT transposes there)."""
                requant_scales(r, pool)
                for ml in range(KHL):
                    for hf in range(2):
                        hmg = pool.tile([128, HL // 2], F32, tag="hmg")
                        kh = pool.tile([128, HL // 2], BF16, tag="kh")
                        if on_vector:
                            # all-VectorE: the ScalarE khT-transpose
                            # stream must never queue behind the AR(7)
                            # wait at the M1->M2 seam
                            nc.vector.tensor_scalar(
                                out=hmg[:, :], in0=h_slice(ml, hf),
                                scalar1=shr[r][:, ml:ml + 1],
                                scalar2=MAGIC, op0=OP.mult, op1=OP.add)
                            nc.vector.tensor_scalar_sub(
                                out=kh[:, :], in0=hmg[:, :], scalar1=MAGIC)
                        else:
                            nc.scalar.activation(
                                out=hmg[:, :], in_=h_slice(ml, hf),
                                func=AF.Copy,
                                scale=shr[r][:, ml:ml + 1], bias=MAGIC)
                            nc.scalar.activation(
                                out=kh[:, :], in_=hmg[:, :],
                                func=AF.Copy, bias=-MAGIC)
                        gp.dma_start(
                            out=a2i[r][ml, :,
                                       hf * (HL // 2):(hf + 1) * (HL // 2)],
                            in_=kh[:, :])
                gp.collective_compute("AllToAll", OP.bypass,
                                      replica_groups=RG,
                                      ins=[a2i[r][:, :, :].opt()],
                                      outs=[a2o[r][:, :, :].opt()])

            # =============== prologue ===============
            with (
                tc.tile_pool(name="wW", bufs=1) as wW,
                tc.tile_pool(name="kxp", bufs=2) as kxp,
                tc.tile_pool(name="xq", bufs=2) as xq,
            ):
                def quant_own(r):
                    """Quantize this core's 128-token chunk of block r,
                    pack kxT + gamma_x(hi/lo bf16) into agx_i[r]."""
                    xt = xq.tile([128, D], F32, tag="x_in")
                    nc.sync.dma_start(out=xt[:, :], in_=Xo[r])
                    gxo = xq.tile([128, 1], F32, tag="gxo")
                    gmx = xq.tile([128, 1], F32, tag="gmx")
                    nc.vector.tensor_reduce(out=gmx[:, :], in_=xt[:, :],
                                            axis=AX.X, op=OP.max,
                                            apply_absolute_value=True)
                    nc.vector.tensor_scalar_max(out=gxo[:, :],
                                                in0=gmx[:, :], scalar1=1e-5)
                    rcp = xq.tile([128, 1], F32, tag="rcpx")
                    nc.vector.reciprocal(out=rcp[:, :], in_=gxo[:, :])
                    sx = xq.tile([128, 1], F32, tag="sx")
                    nc.vector.tensor_scalar_mul(out=sx[:, :], in0=rcp[:, :],
                                                scalar1=127.0)
                    nc.vector.tensor_scalar(out=xt[:, :], in0=xt[:, :],
                                            scalar1=sx[:, :], scalar2=MAGIC,
                                            op0=OP.mult, op1=OP.add)
                    kx = xq.tile([128, D], BF16, tag="kx", bufs=1)
                    nc.scalar.activation(out=kx[:, :], in_=xt[:, :],
                                         func=AF.Copy, bias=-MAGIC)
                    kxT = xq.tile([128, KD, 128], BF16, tag="kxT")
                    nc.scalar.dma_start(out=kxT[:, :, :], in_=kx[:, :],
                                        transpose=True)
                    sc2 = xq.tile([128, 2], BF16, tag="sc2")
                    nc.vector.tensor_scalar_add(out=sc2[:, 0:1],
                                                in0=gxo[:, :], scalar1=0.0)
                    nc.vector.tensor_sub(out=sc2[:, 1:2], in0=gxo[:, :],
                                         in1=sc2[:, 0:1])
                    gp.dma_start(
                        out=agx_i[r][:, 0:KD * 128],
                        in_=kxT[:, :, :].rearrange("p k t -> p (k t)"))
                    gp.dma_start(out=agx_i[r][:, KD * 128:AGW],
                                 in_=sc2[:, :])
                    gp.collective_compute("AllGather", OP.bypass,
                                          replica_groups=RG,
                                          ins=[agx_i[r][:, :].opt()],
                                          outs=[agx_o[r][:, :, :].opt()])

                if True:
                    WgT = wW.tile([128, KD, HL], BF16, tag="WgT")   # 4.2 MB
                    WvT = wW.tile([128, KD, HL], BF16, tag="WvT")   # 4.2 MB
                    with tc.tile_pool(name="stg", bufs=2) as stg:
                        Gst = stg.tile([128, KD, HL], F32, tag="Gst",
                                       bufs=1)                      # 8.4 MB
                        parts = stg.tile([128, 4 * KD], F32, tag="parts",
                                         bufs=1)
                        # gate: load + stage + |w|-accum on ScalarE;
                        # val: rotating load + |w|-reduce on VectorE
                        for c in range(KD):
                            nc.sync.dma_start(out=Gst[:, c, :], in_=Gv[c])
                            scr = stg.tile([128, HL], F32, tag="scr", bufs=1)
                            nc.scalar.activation(
                                out=scr[:, :], in_=Gst[:, c, :],
                                func=AF.Abs,
                                accum_out=parts[:, c:c + 1])
                            wt = stg.tile([128, HL], F32, tag="v_in",
                                          bufs=4)
                            nc.sync.dma_start(out=wt[:, :], in_=Vv[c])
                            nc.vector.tensor_reduce(
                                out=parts[:, KD + c:KD + c + 1],
                                in_=wt[:, :], axis=AX.X, op=OP.add,
                                apply_absolute_value=True)
                        gsum = stg.tile([128, 4], F32, tag="gsum", bufs=1)
                        nc.vector.memset(gsum[:, :], 0.0)
                        for j, sl in enumerate((slice(0, KD),
                                                slice(KD, 2 * KD))):
                            red = stg.tile([128, 1], F32, tag="red")
                            nc.vector.tensor_reduce(out=red[:, :],
                                                    in_=parts[:, sl],
                                                    axis=AX.X, op=OP.add)
                            gp.partition_all_reduce(gsum[:, j:j + 1],
                                                    red[:, :], 128,
                                                    bass_isa.ReduceOp.add)
                        nc.sync.dma_start(out=gv_i[0:1, :],
                                          in_=gsum[0:1, :])
                        gp.collective_compute("AllReduce", OP.add,
                                              replica_groups=RG,
                                              ins=[gv_i[:, :].opt()],
                                              outs=[gv_o[:, :].opt()])
                        # own-token quant for blocks 0,1 + their kx
                        # AllGathers ride right behind AllReduce #1
                        quant_own(0)
                        quant_own(1)
                        # out_w |w|-sum pass rides under AllReduce #1
                        for c in range(KHL):
                            for hf in range(2):
                                wt = stg.tile([128, HL], F32, tag="v_in",
                                              bufs=4)
                                nc.sync.dma_start(
                                    out=wt[:, :],
                                    in_=Ov[c][:, hf * HL:(hf + 1) * HL])
                                col = 2 * KD + 2 * c + hf
                                if hf == 0:
                                    scr = stg.tile([128, HL], F32,
                                                   tag="scr", bufs=1)
                                    nc.scalar.activation(
                                        out=scr[:, :], in_=wt[:, :],
                                        func=AF.Abs,
                                        accum_out=parts[:, col:col + 1])
                                else:
                                    nc.vector.tensor_reduce(
                                        out=parts[:, col:col + 1],
                                        in_=wt[:, :], axis=AX.X, op=OP.add,
                                        apply_absolute_value=True)
                        redo = stg.tile([128, 1], F32, tag="red")
                        nc.vector.tensor_reduce(
                            out=redo[:, :],
                            in_=parts[:, 2 * KD:2 * KD + 2 * KHL],
                            axis=AX.X, op=OP.add)
                        gp.partition_all_reduce(gsum[:, 2:3], redo[:, :],
                                                128, bass_isa.ReduceOp.add)
                        nc.sync.dma_start(out=go_i[0:1, :],
                                          in_=gsum[0:1, :])
                        gp.collective_compute("AllReduce", OP.add,
                                              replica_groups=RG,
                                              ins=[go_i[:, :].opt()],
                                              outs=[go_o[:, :].opt()])
                        # gammas g/v from AllReduce #1
                        g0 = stg.tile([1, 4], F32, tag="g0", bufs=1)
                        nc.sync.dma_start(out=g0[:, :], in_=gv_o[0:1, :])
                        gbc = stg.tile([128, 4], F32, tag="gbc", bufs=1)
                        gp.partition_broadcast(gbc[:, :], g0[:, :])
                        nc.vector.tensor_scalar(out=gam[:, 0:2],
                                                in0=gbc[:, 0:2],
                                                scalar1=1.0 / (H * D),
                                                scalar2=1e-5,
                                                op0=OP.mult, op1=OP.max)
                        for j in range(2):
                            nc.vector.tensor_scalar_mul(
                                out=thr[:, 2 * j:2 * j + 1],
                                in0=gam[:, j:j + 1], scalar1=0.5)
                            nc.vector.tensor_scalar_mul(
                                out=thr[:, 2 * j + 1:2 * j + 2],
                                in0=gam[:, j:j + 1], scalar1=-0.5)
                        s0 = stg.tile([1, N_CORES], F32, tag="s0", bufs=1)
                        nc.sync.dma_start(out=s0[:, :], in_=sel_d.ap())
                        gp.partition_broadcast(selb[:, :], s0[:, :])
                        # ternarize: gate from SBUF (ScalarE), val re-read
                        # (GpSimd queue) + ternarize on VectorE
                        for c in range(KD):
                            tern_act(stg, Gst[:, c, :], WgT[:, c, :],
                                     thr_g, nthr_g)
                            wtv = stg.tile([128, HL], F32, tag="v_in",
                                           bufs=4)
                            nc.sync.dma_start(out=wtv[:, :], in_=Vv[c])
                            tern_dve(stg, wtv[:, :], WvT[:, c, :],
                                     thr_v, nthr_v)

                    def ow_gamma_emit(pool):
                        """gamma_o + thresholds from AllReduce #2."""
                        g1 = pool.tile([1, 4], F32, tag="g1", bufs=1)
                        nc.sync.dma_start(out=g1[:, :], in_=go_o[0:1, :])
                        gb1 = pool.tile([128, 4], F32, tag="gb1", bufs=1)
                        gp.partition_broadcast(gb1[:, :], g1[:, :])
                        nc.vector.tensor_scalar(out=gam[:, 2:3],
                                                in0=gb1[:, 2:3],
                                                scalar1=1.0 / (H * D),
                                                scalar2=1e-5,
                                                op0=OP.mult, op1=OP.max)
                        nc.vector.tensor_scalar_mul(out=thr[:, 4:5],
                                                    in0=gam[:, 2:3],
                                                    scalar1=0.5)
                        nc.vector.tensor_scalar_mul(out=thr[:, 5:6],
                                                    in0=gam[:, 2:3],
                                                    scalar1=-0.5)

                    def ow_tern_batch(pool, c):
                        """Ternarize one of out_w's 8 row-chunks (spread
                        across mm1 blocks, two batches per block)."""
                        wts = []
                        for qf in range(4):
                            wt = pool.tile([128, 512], F32, tag="ow_in",
                                           bufs=4)
                            gp.dma_start(
                                out=wt[:, :],
                                in_=Ov[c][:, qf * 512:(qf + 1) * 512])
                            wts.append(wt)
                        for qf in range(4):
                            tq = pool.tile([128, 512], mybir.dt.int8,
                                           tag="ow_tq")
                            tern = tern_act if qf % 2 == 0 else tern_dve
                            tern(pool, wts[qf][:, :], tq[:, :], thr_o,
                                 nthr_o, w=512)
                            gp.dma_start(
                                out=w2b[c * 128:(c + 1) * 128,
                                        qf * 512:(qf + 1) * 512],
                                in_=tq[:, :])

                    # ===== phase M1: mm1 + silu + requant + A2A =====
                    with (
                        tc.tile_pool(name="m1e", bufs=2) as m1e,
                        tc.tile_pool(name="hbp", bufs=1) as hbp,
                        tc.tile_pool(name="rqm", bufs=2) as rqm,
                    ):
                        hbuf = [hbp.tile([128, KHL, HL], F32,
                                         tag=f"hbuf{b}", name=f"hbuf{b}")
                                for b in range(2)]

                        def h_sb(r):
                            def sl(ml, hf):
                                return hbuf[r % 2][:, ml,
                                                   hf * 512:(hf + 1) * 512]
                            return sl

                        for r in range(N_CORES):
                            hmall = m1e.tile([128, KHL], F32, tag="hmall",
                                             bufs=2, name=f"hmall{r}")
                            for qb in range(4):     # 256-token quarters
                                kxq = kxp.tile([128, KD, 256], BF16,
                                               tag="kxq")
                                for j in range(2):
                                    jj = qb * 2 + j
                                    nc.sync.dma_start(
                                        out=kxq[:, :,
                                                j * 128:(j + 1) * 128],
                                        in_=agx_o[r][jj, :, 0:KD * 128]
                                        .rearrange("p (k t) -> p k t",
                                                   t=128))
                                if qb == 0:
                                    if r + 2 < N_CORES:
                                        quant_own(r + 2)
                                    # reconstruct gamma_x + eviction
                                    # scales for the whole block
                                    gxsc = xq.tile([128, KHL, 2], BF16,
                                                   tag="gxsc")
                                    nc.sync.dma_start(
                                        out=gxsc[:, :, :],
                                        in_=agx_o[r][:, :, KD * 128:AGW]
                                        .rearrange("j p c -> p j c"))
                                    r8 = r * 8
                                    nc.vector.tensor_add(
                                        out=gxall[:, r8:r8 + 8],
                                        in0=gxsc[:, :, 0],
                                        in1=gxsc[:, :, 1])
                                    nc.vector.tensor_scalar(
                                        out=s1a[:, r8:r8 + 8],
                                        in0=gxall[:, r8:r8 + 8],
                                        scalar1=gam[:, 0:1],
                                        scalar2=1.0 / 254.0,
                                        op0=OP.mult, op1=OP.mult)
                                    nc.vector.tensor_scalar(
                                        out=s2a[:, r8:r8 + 8],
                                        in0=gxall[:, r8:r8 + 8],
                                        scalar1=gam[:, 1:2],
                                        scalar2=1.0 / 254.0,
                                        op0=OP.mult, op1=OP.mult)
                                    nc.vector.tensor_mul(
                                        out=s12a[:, r8:r8 + 8],
                                        in0=s1a[:, r8:r8 + 8],
                                        in1=s2a[:, r8:r8 + 8])
                                for j in range(2):
                                    ml = qb * 2 + j
                                    m = r * 8 + ml
                                    hm2 = m1e.tile([128, 2], F32,
                                                   tag="hm2")
                                    ps = [psp.tile([128, 512], F32,
                                                   tag="ps",
                                                   name=f"ps{m}_{i}")
                                          for i in range(4)]
                                    for k in range(KD):
                                        lhsT = kxq[:, k,
                                                   j * 128:(j + 1) * 128]
                                        for i, (w, n) in enumerate(
                                                ((WgT, 0), (WvT, 0),
                                                 (WgT, 1), (WvT, 1))):
                                            nc.tensor.matmul(
                                                ps[i][:, :], lhsT=lhsT,
                                                rhs=w[:, k,
                                                      n * 512:
                                                      (n + 1) * 512],
                                                start=(k == 0),
                                                stop=(k == KD - 1))
                                    for n in range(2):
                                        pg, pv = ps[2 * n], ps[2 * n + 1]
                                        A = m1e.tile([128, 512], F32,
                                                     tag="Asb")
                                        nc.scalar.activation(
                                            out=A[:, :], in_=pg[:, :],
                                            func=AF.Sigmoid,
                                            scale=s1a[:, m:m + 1])
                                        t1 = m1e.tile([128, 512], F32,
                                                      tag="t1sb", bufs=1)
                                        nc.vector.scalar_tensor_tensor(
                                            out=t1[:, :], in0=pg[:, :],
                                            scalar=s12a[:, m:m + 1],
                                            in1=A[:, :],
                                            op0=OP.mult, op1=OP.mult)
                                        hs_ap = hbuf[r % 2][
                                            :, ml, n * 512:(n + 1) * 512]
                                        nc.vector.tensor_mul(out=hs_ap,
                                                             in0=pv[:, :],
                                                             in1=t1[:, :])
                                        nc.vector.tensor_reduce(
                                            out=hm2[:, n:n + 1],
                                            in_=hs_ap, axis=AX.X,
                                            op=OP.max,
                                            apply_absolute_value=True)
                                    nc.vector.tensor_max(
                                        out=hmall[:, ml:ml + 1],
                                        in0=hm2[:, 0:1], in1=hm2[:, 1:2])
                                if r == 0 and qb == 2:
                                    ow_gamma_emit(rqm)
                                # out_w chunks 0..7 spread over blocks
                                # 0..4; the last lands at block 4 so the
                                # w2b data-dep pins the 32MB AllGather
                                # to ~block 4's end on the cc stream
                                owc = {(0, 3): 0, (1, 1): 1, (1, 2): 2,
                                       (2, 1): 3, (2, 2): 4, (3, 1): 5,
                                       (3, 2): 6, (4, 1): 7}.get((r, qb))
                                if owc is not None:
                                    ow_tern_batch(rqm, owc)
                                    if owc % 2 == 1:
                                        # this w2b quarter is complete:
                                        # its AllGather chunk can go (the
                                        # trigger must queue BEHIND its
                                        # producers on the GpSimd FIFO)
                                        q = owc // 2
                                        gp.collective_compute(
                                            "AllGather", OP.bypass,
                                            replica_groups=RG,
                                            ins=[w2b[q * 256:(q + 1) * 256,
                                                     :].opt()],
                                            outs=[w2gq[q][:, :].opt()])
                                if r >= 1 and qb == 3:
                                    # delayed requant of block r-1
                                    requant_emit(r - 1, rqm, h_sb(r - 1))
                            nc.sync.dma_start(
                                out=arh_i[r][0, :]
                                .rearrange("(ml p) -> p ml", p=128),
                                in_=hmall[:, :])
                            gp.collective_compute(
                                "AllReduce", OP.max, replica_groups=RG,
                                ins=[arh_i[r][:, :].opt()],
                                outs=[arh_o[r][:, :].opt()])
                        # block-7 requant, HERE inside M1 scope: reads
                        # hbuf[1] directly (no DRAM spill) and at mm1's
                        # end there is nothing left to head-of-line
                        # block. VectorE mode: M2's ScalarE/Sync carry
                        # the khT transposes.
                        requant_emit(RLAST, rqm, h_sb(RLAST),
                                     on_vector=True)

            # ================= phase M2: mm2, two r-groups =================
            with (
                tc.tile_pool(name="m2k", bufs=1) as m2k,
                tc.tile_pool(name="m2w", bufs=12) as m2w,
                tc.tile_pool(name="m2o", bufs=6) as m2o,
                tc.tile_pool(name="rq2", bufs=2) as rq2,
            ):
                khTg = [m2k.tile([128, KH, 128], BF16, tag=f"khTg{r}",
                                 name=f"khTg{r}") for r in range(N_CORES)]
                Woq = [w2gq[q][:, :].rearrange("(n p) d -> n p d", p=128)
                       for q in range(4)]

                def Wo(k):     # global hidden row-chunk k of ternary out_w
                    return Woq[(k % 8) // 2][(k // 8) * 2 + (k % 2)]

                Outv = out_d.ap().rearrange("(r p) d -> r p d", p=128)

                def khTg_fill(r):
                    """khT for block r straight from the A2A output:
                    8 transpose-DMAs (Scalar HWDGE only — concurrent
                    transposes from two queues corrupt data)."""
                    for j in range(N_CORES):
                        nc.scalar.dma_start(
                            out=khTg[r][:, j * KHL:(j + 1) * KHL, :],
                            in_=a2o[r][j], transpose=True)

                # khT transposes j-major over r0..6 so the first k-rows
                # of every block land first; r=7 behind its A2A
                for j in range(N_CORES):
                    for r in range(N_CORES - 1):
                        nc.scalar.dma_start(
                            out=khTg[r][:, j * KHL:(j + 1) * KHL, :],
                            in_=a2o[r][j], transpose=True)
                khTg_fill(RLAST)

                def evict(po_r, r, dcol):
                    ot = m2o.tile([128, 512], F32, tag="ot")
                    nc.scalar.activation(out=ot[:, :], in_=po_r[:, :],
                                         func=AF.Copy,
                                         scale=sosel[r][:, :])
                    # Sync, NOT GpSimd: an eviction ahead of the a2i(7)
                    # writes on the GpSimd FIFO would delay the AllToAll
                    # trigger behind whole matmul passes
                    nc.sync.dma_start(
                        out=Outv[r][:, dcol * 512:(dcol + 1) * 512],
                        in_=ot[:, :])

                def col_pass(grp, dcol):
                    """One dcol pass over up to 8 token-blocks: one 1KB
                    weight tile per len(grp) matmuls, weights read once
                    per dcol."""
                    po = [psp.tile([128, 512], F32, tag="ps",
                                   name=f"po{grp[0]}_{dcol}_{i}")
                          for i in range(len(grp))]
                    for k in range(KH):
                        w2ti = m2w.tile([128, 512], mybir.dt.int8,
                                        tag="w2ti")
                        nc.sync.dma_start(
                            out=w2ti[:, :],
                            in_=Wo(k)[:, dcol * 512:(dcol + 1) * 512])
                        w2t = m2w.tile([128, 512], BF16, tag="w2t")
                        nc.vector.tensor_scalar_add(out=w2t[:, :],
                                                    in0=w2ti[:, :],
                                                    scalar1=0.0)
                        for i, r in enumerate(grp):
                            nc.tensor.matmul(
                                po[i][:, :],
                                lhsT=khTg[r][:, k, :],
                                rhs=w2t[:, :],
                                start=(k == 0), stop=(k == KH - 1))
                    for i, r in enumerate(grp):
                        evict(po[i], r, dcol)

                # dcol0 without r=7 (its AllToAll is still in flight),
                # then full passes, then the small r=7/dcol0 make-up
                col_pass(list(range(7)), 0)
                for dcol in range(1, 4):
                    col_pass(list(range(8)), dcol)
                col_pass([RLAST], 0)


_NC_CACHE = {}


def _get_nc():
    if "nc" not in _NC_CACHE:
        _NC_CACHE["nc"] = _build()
    return _NC_CACHE["nc"]


def kernel(x, gate_w, gate_b, val_w, val_b, out_w, out_b, _trace=False):
    x = np.ascontiguousarray(np.asarray(x), dtype=np.float32)
    gate_w = np.asarray(gate_w, dtype=np.float32)
    val_w = np.asarray(val_w, dtype=np.float32)
    out_w = np.asarray(out_w, dtype=np.float32)
    gate_b = np.asarray(gate_b)
    val_b = np.asarray(val_b)
    out_b = np.asarray(out_b)
    assert not np.any(gate_b) and not np.any(val_b), (
        "device kernel folds silu(y+b) with b=0; nonzero gate/val bias "
        "not supported")

    orig_shape = x.shape
    xf = x.reshape(-1, x.shape[-1])
    assert xf.shape == (T, D) and gate_w.shape == (H, D)
    assert val_w.shape == (H, D) and out_w.shape == (D, H)
    xi = xf.reshape(N_CORES, N_CORES, 128, D)        # [r, i, p, d]

    nc = _get_nc()
    in_maps = []
    for i in range(N_CORES):
        sel = np.zeros((1, N_CORES), np.float32)
        sel[0, i] = 1.0
        in_maps.append({
            "x_own": np.ascontiguousarray(xi[:, i].reshape(TL, D)),
            "gwT": np.ascontiguousarray(gate_w[i * HL:(i + 1) * HL, :].T),
            "vwT": np.ascontiguousarray(val_w[i * HL:(i + 1) * HL, :].T),
            "owT": np.ascontiguousarray(out_w[:, i * HL:(i + 1) * HL].T),
            "sel8": sel,
        })
    res = run_bass_kernel_spmd(nc, in_maps, core_ids=list(range(N_CORES)),
                               trace=_trace)
    # core i owns tokens r*1024 + i*128 + [0,128) for r in 0..7
    out = np.empty((T, D), np.float32)
    ov = out.reshape(N_CORES, N_CORES, 128, D)       # [r, i, p, d]
    for i in range(N_CORES):
        ov[:, i] = res.results[i]["out"].reshape(N_CORES, 128, D)
    out = out + out_b[None, :].astype(np.float32)
    kernel._last_results = res
    return out.reshape(orig_shape)
